# revision 11
# baseline (speedup 1.0000x reference)
"""2-layer GCN (gnn_message_passing) on 8 Trainium2 NeuronCores.

Strategy (v2):
  - Fold the symmetric degree normalization into per-node scaling:
      msg_e = dinv[src]*dinv[dst]*h[src]  =>  agg = dinv * A_sum(dinv * h)
    so aggregation is a pure unweighted gather + segment-sum.
  - Node-partition the graph over 8 cores (12544 dst nodes per core).
  - Each core computes h' = (x_shard @ W^T + b) * dinv for its shard,
    AllGathers the full h' table (f32 rows of 64 floats = 256B), then
    aggregates its own dst tiles with batched dma_gather (int16 indices
    -> 4 table "classes" of <=25090 rows each) + strided vector reduces.
  - Self-loop terms never enter the gather streams: each dst tile's
    accumulator is initialized with dinv^2 * h from the SBUF-resident
    h' tiles.
  - Dst nodes are clustered into tiles by their (max, argmax, full
    vector) in-neighbor class profile so the per-(tile,class) padded
    slot count tracks the actual in-degree profile (~22% padding).
  - Layer-1 epilogue (relu fused into the dinv scale, transpose,
    matmul2 with bias folded in via a ones row) is pipelined per tile
    group under the remaining gathers, so AllGather-2 launches right
    after the last layer-1 reduce. The layer-2 softmax pipeline (exp
    with accumulate, one deferred Ln over all tiles, fused
    subtract+store) runs under the layer-2 gathers.
"""

import sys
import numpy as np

sys.path.insert(0, "/opt/trn_rl_repo")

N = 100000
E = 1600000
NFEAT, NHID, NCLASS = 128, 64, 40
NCORES = 8
CPN = 12544            # dst nodes per core (98 tiles of 128)
BLK = CPN + 1          # AllGather block rows per core (+1 zero pad row)
NT = CPN // 128        # 98 aggregation tiles per core
NT1 = NT + 1           # 99 phase-1 tiles (covers the pad row)
XCOLS = NT1 * 128      # 12672
NCLS = 4
CLS_ROWS = 2 * BLK     # 25090 table rows per class (= 2 core blocks)
TBL = NCORES * BLK     # 100360
PAD_LOCAL = CPN        # local index of the zero row within a class
GB_SLOTS = 64          # gather ring buffer slots per call
GB_BUFS = 4
ORING = 4              # output store ring
F = 64                 # table row width (f32) = 256B
NXCH = 4               # xT upload chunks


def _host_prep(edge_index):
    """Returns permutation/class layout + per-core padded gather streams."""
    src = edge_index[0].astype(np.int64)
    dst = edge_index[1].astype(np.int64)
    deg = np.bincount(src, minlength=N) + 1      # +1: self-loop
    dinv = (1.0 / np.sqrt(deg.astype(np.float64))).astype(np.float32)

    # ---- greedy class assignment of sources (balance each dst's in-nbrs) ----
    order_e = np.argsort(src, kind="stable")
    d_sorted = dst[order_e]
    sptr = np.searchsorted(src[order_e], np.arange(N + 1))
    cap = NCORES * CPN // NCLS                    # 25088 real nodes max per class
    cnt = np.zeros((N, NCLS), np.int32)
    cls = np.full(N, -1, np.int8)
    szs = np.zeros(NCLS, np.int64)
    outdeg = np.bincount(src, minlength=N)
    sorder = np.argsort(-outdeg, kind="stable")
    for s in sorder:
        dd = d_sorted[sptr[s]:sptr[s + 1]]
        sc = (4.0 ** cnt[dd, :]).sum(0)
        sc = sc + (szs >= cap) * 1e30
        c = int(sc.argmin())
        cls[s] = c
        szs[c] += 1
        cnt[dd, c] += 1

    # ---- node -> (core, position): cluster similar in-profiles per tile ----
    blocks = []                                   # per core: array of node ids (-1 = dummy)
    for c in range(NCLS):
        nodes_c = np.flatnonzero(cls == c)
        cc = cnt[nodes_c]
        order = np.lexsort((cc[:, 3], cc[:, 2], cc[:, 1], cc[:, 0],
                            cc.argmax(1), cc.max(1)))
        nodes_c = nodes_c[order]
        a = np.full(CPN, -1, np.int64)
        b = np.full(CPN, -1, np.int64)
        a[: (len(nodes_c) + 1) // 2] = nodes_c[0::2]
        b[: len(nodes_c) // 2] = nodes_c[1::2]
        blocks.append(a)
        blocks.append(b)

    # table row of node n (within the AllGathered table)
    row = np.full(N, -1, np.int64)
    for k in range(NCORES):
        blk = blocks[k]
        real = blk >= 0
        row[blk[real]] = k * BLK + np.flatnonzero(real)

    # ---- per-(core,tile,class,partition) in-neighbor counts ----
    dcore = np.empty(N, np.int64)
    dpos = np.empty(N, np.int64)
    for k in range(NCORES):
        blk = blocks[k]
        real = blk >= 0
        dcore[blk[real]] = k
        dpos[blk[real]] = np.flatnonzero(real)
    ecore = dcore[dst]
    epos = dpos[dst]
    etile = epos // 128
    epart = epos % 128
    ecls = cls[src].astype(np.int64)
    esrow = row[src] - ecls * CLS_ROWS            # class-local table row (0..25089)
    assert esrow.min() >= 0 and esrow.max() < CLS_ROWS

    key = ((ecore * NT + etile) * NCLS + ecls) * 128 + epart
    eorder = np.argsort(key, kind="stable")
    key_s = key[eorder]
    esrow_s = esrow[eorder]
    counts = np.bincount(key_s, minlength=NCORES * NT * NCLS * 128)
    counts = counts.reshape(NCORES, NT, NCLS, 128)
    kmax = counts.max(axis=(0, 3))                # common K per (tile,class), max over cores
    kmax = np.maximum(kmax, 1)                    # avoid zero-length segments

    # ---- call grouping: consecutive tiles, per-class slot sum <= GB_SLOTS ----
    groups = []
    cur = []
    for t in range(NT):
        trial = cur + [t]
        if cur and max(kmax[trial, c].sum() for c in range(NCLS)) > GB_SLOTS:
            groups.append(cur)
            cur = [t]
        else:
            cur = trial
        if kmax[t].max() > GB_SLOTS:
            raise RuntimeError("single tile exceeds gather buffer")
    groups.append(cur)

    # calls: (class, tiles, seg_offsets, nslots); same order for both layers
    calls = []
    for g in groups:
        for c in range(NCLS):
            offs = np.concatenate([[0], np.cumsum(kmax[g, c])])
            calls.append((c, list(g), offs[:-1].tolist(), int(offs[-1])))

    total_slots = sum(nsl for (_, _, _, nsl) in calls)

    # ---- build per-core int16 index stream in call order (vectorized) ----
    flat_counts = counts.reshape(-1)
    starts = np.concatenate([[0], np.cumsum(flat_counts)])[:-1].reshape(NCORES, NT, NCLS, 128)
    epart_s = epart[eorder]

    streams = []
    for k in range(NCORES):
        stream = np.full(total_slots * 128, PAD_LOCAL, np.int16)
        base = 0
        for (c, tiles, offs, nsl) in calls:
            for t, off in zip(tiles, offs):
                cnts = counts[k, t, c]            # [128]
                st = starts[k, t, c]              # [128]
                tot = int(cnts.sum())
                if tot:
                    sl = slice(st[0], st[0] + tot)
                    parts = epart_s[sl]
                    jj = np.arange(tot) - np.repeat(st - st[0], cnts)
                    stream[base + (off + jj) * 128 + parts] = esrow_s[sl].astype(np.int16)
            base += nsl * 128
        # wrap for dma_gather: idxs[p, j] = stream[j*16 + p%16]
        wrapped = stream.reshape(-1, 16).T        # [16, cols]
        streams.append(np.tile(wrapped, (8, 1)))  # [128, cols]

    meta = dict(blocks=blocks, row=row, dinv=dinv, calls=calls, groups=groups,
                total_slots=total_slots, kmax=kmax)
    return meta, streams


def _build_program(calls, groups):
    import concourse.bacc as bacc
    import concourse.bass as bass
    from concourse import mybir
    from concourse.library_config import mlp
    from contextlib import ExitStack

    AF = mybir.ActivationFunctionType
    OP = mybir.AluOpType
    nc = bacc.Bacc("TRN2", target_bir_lowering=False, debug=False)

    NCALLS = len(calls)
    total_slots = sum(nsl for (_, _, _, nsl) in calls)
    COLS = total_slots * 8                        # int16 idx cols per partition
    # tile-aligned upload chunk boundaries (in cols)
    tchunk = [0, 25, 50, 75, NT1]
    xbound = [t * 128 for t in tchunk]

    xT = nc.declare_dram_parameter("xT", [128, XCOLS], mybir.dt.float32, isOutput=False)
    idxp = nc.declare_dram_parameter("idx", [128, COLS], mybir.dt.int16, isOutput=False)
    w1t = nc.declare_dram_parameter("w1t", [128, NHID], mybir.dt.float32, isOutput=False)
    w2p = nc.declare_dram_parameter("w2p", [NHID + 1, F], mybir.dt.float32, isOutput=False)
    b1b = nc.declare_dram_parameter("b1b", [128, NHID], mybir.dt.float32, isOutput=False)
    dvc = nc.declare_dram_parameter("dvc", [128, NT1], mybir.dt.float32, isOutput=False)
    idn = nc.declare_dram_parameter("idn", [128, 128], mybir.dt.float32, isOutput=False)
    zro = nc.declare_dram_parameter("zro", [1, F], mybir.dt.float32, isOutput=False)
    outp = nc.declare_dram_parameter("out", [CPN, NCLASS], mybir.dt.float32, isOutput=True)

    h1_own = nc.dram_tensor("h1_own", [XCOLS, F], mybir.dt.float32)
    h2_own = nc.dram_tensor("h2_own", [BLK, F], mybir.dt.float32)
    h1_full = nc.dram_tensor("h1_full", [TBL, F], mybir.dt.float32, addr_space="Shared")
    h2_full = nc.dram_tensor("h2_full", [TBL, F], mybir.dt.float32, addr_space="Shared")

    with ExitStack() as stack:
        ec = stack.enter_context
        block = ec(nc.Block())
        xT_sb = ec(nc.sbuf_tensor("xT_sb", [128, XCOLS], mybir.dt.float32))
        idx_sb = ec(nc.sbuf_tensor("idx_sb", [128, COLS], mybir.dt.int16))
        w1t_sb = ec(nc.sbuf_tensor("w1t_sb", [128, NHID], mybir.dt.float32))
        w2p_sb = ec(nc.sbuf_tensor("w2p_sb", [NHID + 1, F], mybir.dt.float32))
        b1b_sb = ec(nc.sbuf_tensor("b1b_sb", [128, NHID], mybir.dt.float32))
        dvc_sb = ec(nc.sbuf_tensor("dvc_sb", [128, NT1], mybir.dt.float32))
        idn_sb = ec(nc.sbuf_tensor("idn_sb", [128, 128], mybir.dt.float32))
        zro_sb = ec(nc.sbuf_tensor("zro_sb", [1, F], mybir.dt.float32))
        gbuf = ec(nc.sbuf_tensor("gbuf", [128, GB_BUFS, GB_SLOTS, F], mybir.dt.float32))
        agg = ec(nc.sbuf_tensor("agg", [128, NT, F], mybir.dt.float32))
        hs = ec(nc.sbuf_tensor("hs", [128, NT1, F], mybir.dt.float32))
        x2T = ec(nc.sbuf_tensor("x2T", [NHID + 1, 2, 128], mybir.dt.float32))
        osb = ec(nc.sbuf_tensor("osb", [128, ORING, NCLASS], mybir.dt.float32))
        tmp = ec(nc.sbuf_tensor("tmp", [128, F], mybir.dt.float32))
        tmp2 = ec(nc.sbuf_tensor("tmp2", [128, NCLASS], mybir.dt.float32))
        lse = ec(nc.sbuf_tensor("lse", [128, NT], mybir.dt.float32))
        lnl = ec(nc.sbuf_tensor("lnl", [128, NT], mybir.dt.float32))
        ph1 = ec(nc.psum_tensor("ph1", [128, 4, NHID], mybir.dt.float32))
        pT = ec(nc.psum_tensor("pT", [NHID, 2, 512], mybir.dt.float32))
        p2 = ec(nc.psum_tensor("p2", [128, 2, 512], mybir.dt.float32))
        sems = {n: ec(nc.semaphore(n)) for n in [
            "s_in", "s_xin", "s_idx", "s_mm1", "s_ep1", "s_st1", "s_cc", "s_g",
            "s_red", "s_x2", "s_tp", "s_cp", "s_mm2", "s_ep2", "s_st2", "s_z",
            "s_ln", "s_sm", "s_out"]}
        (s_in, s_xin, s_idx, s_mm1, s_ep1, s_st1, s_cc, s_g, s_red, s_x2, s_tp,
         s_cp, s_mm2, s_ep2, s_st2, s_z, s_ln, s_sm, s_out) = (
            sems[n] for n in ["s_in", "s_xin", "s_idx", "s_mm1", "s_ep1",
                              "s_st1", "s_cc", "s_g", "s_red", "s_x2", "s_tp",
                              "s_cp", "s_mm2", "s_ep2", "s_st2", "s_z", "s_ln",
                              "s_sm", "s_out"])

        # ---------------- sync engine: uploads + stores ----------------
        @block.sync
        def _(se: bass.BassEngine):
            se.dma_start(w1t_sb[:], w1t[:]).then_inc(s_in, 16)       # s_in 16
            se.dma_start(b1b_sb[:], b1b[:]).then_inc(s_in, 16)       # s_in 32
            se.dma_start(dvc_sb[:], dvc[:]).then_inc(s_in, 16)       # s_in 48
            for c in range(NXCH):
                se.dma_start(xT_sb[:, xbound[c]:xbound[c + 1]],
                             xT[:, xbound[c]:xbound[c + 1]]).then_inc(s_xin, 16)
            se.dma_start(idn_sb[:], idn[:]).then_inc(s_in, 16)       # s_in 64
            se.dma_start(w2p_sb[:], w2p[:]).then_inc(s_in, 16)       # s_in 80
            se.dma_start(zro_sb[:], zro[:]).then_inc(s_in, 16)       # s_in 96
            # h2 zero pad row (zro_sb upload must have landed first)
            se.wait_ge(s_in, 96)
            se.dma_start(h2_own[CPN:CPN + 1, :], zro_sb[:]).then_inc(s_st2, 16)
            # phase-1 stores
            for t in range(NT1):
                se.wait_ge(s_ep1, t + 1)
                se.dma_start(h1_own[t * 128:(t + 1) * 128, :], hs[:, t, :]).then_inc(s_st1, 16)
            # layer-2 h2' stores
            for t in range(NT):
                se.wait_ge(s_ep2, t + 1)
                se.dma_start(h2_own[t * 128:(t + 1) * 128, :], hs[:, t, :]).then_inc(s_st2, 16)
            # output stores
            for t in range(NT):
                se.wait_ge(s_sm, t + 1)
                se.dma_start(outp[t * 128:(t + 1) * 128, :], osb[:, t % ORING, :]).then_inc(s_out, 16)
            se.wait_ge(s_out, 16 * NT)

        # ---------------- gpsimd: idx upload, collectives, gathers ----------------
        @block.gpsimd
        def _(g: bass.BassGpSimd):
            g.load_library(mlp)
            g.dma_start(idx_sb[:], idxp[:]).then_inc(s_idx, 16)
            g.wait_ge(s_idx, 16)
            for layer in (0, 1):
                tblt = h1_full if layer == 0 else h2_full
                if layer == 0:
                    g.wait_ge(s_st1, 16 * NT1)
                    g.collective_compute(
                        "AllGather", mybir.AluOpType.bypass,
                        replica_groups=[list(range(NCORES))],
                        ins=[h1_own[0:BLK, :].opt()],
                        outs=[h1_full[:, :].opt()],
                    ).then_inc(s_cc)
                    g.wait_ge(s_cc, 1)
                else:
                    g.wait_ge(s_st2, 16 * (NT + 1))
                    g.collective_compute(
                        "AllGather", mybir.AluOpType.bypass,
                        replica_groups=[list(range(NCORES))],
                        ins=[h2_own[:, :].opt()],
                        outs=[h2_full[:, :].opt()],
                    ).then_inc(s_cc)
                    g.wait_ge(s_cc, 2)
                off = 0
                for j, (c, tiles, offs, nsl) in enumerate(calls):
                    gj = layer * NCALLS + j
                    if gj >= GB_BUFS:
                        g.wait_ge(s_red, gj - GB_BUFS + 1)
                    nidx = nsl * 128
                    g.dma_gather(
                        gbuf[:, gj % GB_BUFS, :nsl, :],
                        tblt[c * CLS_ROWS:(c + 1) * CLS_ROWS, :],
                        idx_sb[:, off * 8:(off + nsl) * 8],
                        nidx, nidx, F,
                        single_packet=False,
                    ).then_inc(s_g, 16)
                    off += nsl
                off = 0

        # ---------------- tensor engine ----------------
        @block.tensor
        def _(te):
            te.wait_ge(s_in, 16)
            for t in range(NT1):
                te.wait_ge(s_xin, 16 * (t // 25 + 1))
                if t >= 4:
                    te.wait_ge(s_ep1, t - 3)
                te.matmul(ph1[:, t % 4, :], xT_sb[:, t * 128:(t + 1) * 128], w1t_sb[:]).then_inc(s_mm1)
            # layer-2: software-pipelined transpose / matmul2
            te.wait_ge(s_in, 80)
            for t in range(NT):
                te.wait_ge(s_x2, t + 1)
                if t >= 2:
                    te.wait_ge(s_cp, t - 1)
                te.transpose(pT[:, t % 2, :128], agg[:, t, :], idn_sb[:]).then_inc(s_tp)
                if t >= 1:
                    te.wait_ge(s_cp, t)
                    if t >= 3:
                        te.wait_ge(s_ep2, t - 2)
                    te.matmul(p2[:, (t - 1) % 2, :F], x2T[:, (t - 1) % 2, :], w2p_sb[:]).then_inc(s_mm2)
            te.wait_ge(s_cp, NT)
            te.wait_ge(s_ep2, NT - 2)
            te.matmul(p2[:, (NT - 1) % 2, :F], x2T[:, (NT - 1) % 2, :], w2p_sb[:]).then_inc(s_mm2)

        # ---------------- vector engine ----------------
        @block.vector
        def _(v: bass.BassVectorEngine):
            v.wait_ge(s_in, 48)
            v.memset(x2T[NHID:NHID + 1, :, :], 1.0)   # bias row for matmul2
            # phase 1: h1' tiles (persistent)
            for t in range(NT1):
                v.wait_ge(s_mm1, t + 1)
                v.tensor_add(hs[:, t, :], ph1[:, t % 4, :], b1b_sb[:])
                v.tensor_scalar(out=hs[:, t, :], in0=hs[:, t, :],
                                scalar1=dvc_sb[:, t:t + 1], scalar2=None,
                                op0=OP.mult).then_inc(s_ep1)
            # self-loop agg init (runs under AllGather-1)
            for t in range(NT):
                v.tensor_scalar(out=agg[:, t, :], in0=hs[:, t, :],
                                scalar1=dvc_sb[:, t:t + 1], scalar2=None,
                                op0=OP.mult)

            def _ep2(t):
                # h2' tile + layer-2 self-loop agg init (overwrites x2 in agg)
                v.wait_ge(s_mm2, t + 1)
                v.tensor_scalar(out=hs[:, t, :], in0=p2[:, t % 2, :F],
                                scalar1=dvc_sb[:, t:t + 1], scalar2=None,
                                op0=OP.mult).then_inc(s_ep2)
                v.tensor_scalar(out=agg[:, t, :], in0=p2[:, t % 2, :F],
                                scalar1=dvc_sb[:, t:t + 1], scalar2=dvc_sb[:, t:t + 1],
                                op0=OP.mult, op1=OP.mult)

            for layer in (0, 1):
                for gi, gtiles in enumerate(groups):
                    for c in range(NCLS):
                        j = gi * NCLS + c
                        (_, tiles, offs, nsl) = calls[j]
                        gj = layer * NCALLS + j
                        v.wait_ge(s_g, 16 * (gj + 1))
                        for ti, t in enumerate(tiles):
                            off = offs[ti]
                            K = (offs[ti + 1] - offs[ti]) if ti + 1 < len(tiles) else nsl - offs[ti]
                            seg = gbuf[:, gj % GB_BUFS, off:off + K, :].rearrange("p k f -> p f k")
                            v.tensor_reduce(tmp[:], seg, axis=mybir.AxisListType.X, op=OP.add)
                            ta = v.tensor_add(agg[:, t, :], agg[:, t, :], tmp[:])
                            if layer == 1 and c == NCLS - 1:
                                ta.then_inc(s_z)
                        v.nop().then_inc(s_red, 1)
                    if layer == 0:
                        # x2 = relu(dinv * agg), then pipelined transpose-copy/ep2
                        for t in gtiles:
                            v.tensor_scalar(out=agg[:, t, :], in0=agg[:, t, :],
                                            scalar1=dvc_sb[:, t:t + 1], scalar2=0.0,
                                            op0=OP.mult, op1=OP.max).then_inc(s_x2)
                        for t in gtiles:
                            v.wait_ge(s_tp, t + 1)
                            v.tensor_copy(x2T[:NHID, t % 2, :], pT[:, t % 2, :128]).then_inc(s_cp)
                            if t >= 1:
                                _ep2(t - 1)
                    else:
                        # final: out = dinv*agg - ln(sum exp), per group
                        for t in gtiles:
                            v.wait_ge(s_ln, gi + 1)
                            if t >= ORING:
                                v.wait_ge(s_out, 16 * (t - ORING + 1))
                            v.tensor_scalar(out=osb[:, t % ORING, :], in0=agg[:, t, :NCLASS],
                                            scalar1=dvc_sb[:, t:t + 1], scalar2=lnl[:, t:t + 1],
                                            op0=OP.mult, op1=OP.subtract).then_inc(s_sm)
                if layer == 0:
                    _ep2(NT - 1)

        # ---------------- scalar engine: exp accumulate + per-group Ln ----------------
        @block.scalar
        def _(sc):
            sc.wait_ge(s_in, 48)
            for gi, gtiles in enumerate(groups):
                for t in gtiles:
                    sc.wait_ge(s_z, t + 1)
                    sc.activation(tmp2[:], agg[:, t, :NCLASS], AF.Exp,
                                  scale=dvc_sb[:, t:t + 1],
                                  accum_out=lse[:, t:t + 1])
                t0, t1 = gtiles[0], gtiles[-1] + 1
                sc.activation(lnl[:, t0:t1], lse[:, t0:t1], AF.Ln).then_inc(s_ln)

    nc.compile()
    return nc


_LAST_NC = None


def kernel(x, W1, b1, W2, b2, edge_index):
    global _LAST_NC
    from concourse.bass_utils import run_bass_kernel_spmd

    x = np.asarray(x)
    W1 = np.asarray(W1); b1 = np.asarray(b1)
    W2 = np.asarray(W2); b2 = np.asarray(b2)
    edge_index = np.asarray(edge_index)

    meta, streams = _host_prep(edge_index)
    calls = meta["calls"]
    groups = meta["groups"]
    nc = _build_program(calls, groups)
    _LAST_NC = nc

    dinv = meta["dinv"]
    blocks = meta["blocks"]
    ident = np.eye(128, dtype=np.float32)
    w1t_np = W1.T.astype(np.float32).copy()                      # [128,64]
    w2p_np = np.zeros((NHID + 1, F), np.float32)
    w2p_np[:NHID, :NCLASS] = W2.T
    w2p_np[NHID, :NCLASS] = b2                                   # bias row
    b1b_np = np.tile(b1.astype(np.float32), (128, 1))

    in_maps = []
    for k in range(NCORES):
        blk = blocks[k]
        real = blk >= 0
        xTk = np.zeros((128, XCOLS), np.float32)
        dvk = np.zeros(XCOLS, np.float32)
        idxs = np.flatnonzero(real)
        xcols = np.zeros((XCOLS, NFEAT), np.float32)
        xcols[idxs] = x[blk[idxs]]
        xTk[:, :] = xcols.T
        dvk[idxs] = dinv[blk[idxs]]
        dvc_np = dvk.reshape(NT1, 128).T.copy()                  # [128, NT1]
        in_maps.append({
            "xT": xTk, "idx": streams[k], "w1t": w1t_np, "w2p": w2p_np,
            "b1b": b1b_np, "dvc": dvc_np, "idn": ident,
            "zro": np.zeros((1, F), np.float32),
        })

    res = run_bass_kernel_spmd(nc, in_maps, list(range(NCORES)))

    out = np.empty((N, NCLASS), np.float32)
    for k in range(NCORES):
        blk = blocks[k]
        real = blk >= 0
        out[blk[real]] = res.results[k]["out"][np.flatnonzero(real)]
    return out


# revision 23
# speedup vs baseline: 1.0546x; 1.0546x over previous
"""2-layer GCN (gnn_message_passing) on 8 Trainium2 NeuronCores.

Strategy (v2):
  - Fold the symmetric degree normalization into per-node scaling:
      msg_e = dinv[src]*dinv[dst]*h[src]  =>  agg = dinv * A_sum(dinv * h)
    so aggregation is a pure unweighted gather + segment-sum.
  - Node-partition the graph over 8 cores (12544 dst nodes per core).
  - Each core computes h' = (x_shard @ W^T + b) * dinv for its shard,
    AllGathers the full h' table (f32 rows of 64 floats = 256B), then
    aggregates its own dst tiles with batched dma_gather (int16 indices
    -> 4 table "classes" of <=25090 rows each) + strided vector reduces.
  - Self-loop terms never enter the gather streams: each dst tile's
    accumulator is initialized with dinv^2 * h from the SBUF-resident
    h' tiles.
  - Dst nodes are clustered into tiles by their (max, argmax, full
    vector) in-neighbor class profile so the per-(tile,class) padded
    slot count tracks the actual in-degree profile (~22% padding).
  - Layer-1 epilogue (relu fused into the dinv scale, transpose,
    matmul2 with bias folded in via a ones row) is pipelined per tile
    group under the remaining gathers, so AllGather-2 launches right
    after the last layer-1 reduce. The layer-2 softmax pipeline (exp
    with accumulate, one deferred Ln over all tiles, fused
    subtract+store) runs under the layer-2 gathers.
"""

import sys
import numpy as np

sys.path.insert(0, "/opt/trn_rl_repo")

N = 100000
E = 1600000
NFEAT, NHID, NCLASS = 128, 64, 40
NCORES = 8
CPN = 12544            # dst nodes per core (98 tiles of 128)
BLK = CPN + 1          # AllGather block rows per core (+1 zero pad row)
NT = CPN // 128        # 98 aggregation tiles per core
NT1 = NT + 1           # 99 phase-1 tiles (covers the pad row)
XCOLS = NT1 * 128      # 12672
NCLS = 4
CLS_ROWS = 2 * BLK     # 25090 table rows per class (= 2 core blocks)
TBL = NCORES * BLK     # 100360
PAD_LOCAL = CPN        # local index of the zero row within a class
GB_SLOTS = 64          # gather ring buffer slots per call
GB_BUFS = 4
ORING = 4              # output store ring
F = 64                 # table row width (f32) = 256B
NXCH = 4               # xT upload chunks


def _host_prep(edge_index):
    """Returns permutation/class layout + per-core padded gather streams."""
    src = edge_index[0].astype(np.int64)
    dst = edge_index[1].astype(np.int64)
    deg = np.bincount(src, minlength=N) + 1      # +1: self-loop
    dinv = (1.0 / np.sqrt(deg.astype(np.float64))).astype(np.float32)

    # ---- greedy class assignment of sources (balance each dst's in-nbrs) ----
    order_e = np.argsort(src, kind="stable")
    d_sorted = dst[order_e]
    sptr = np.searchsorted(src[order_e], np.arange(N + 1))
    cap = NCORES * CPN // NCLS                    # 25088 real nodes max per class
    cnt = np.zeros((N, NCLS), np.int32)
    cls = np.full(N, -1, np.int8)
    szs = np.zeros(NCLS, np.int64)
    outdeg = np.bincount(src, minlength=N)
    sorder = np.argsort(-outdeg, kind="stable")
    for s in sorder:
        dd = d_sorted[sptr[s]:sptr[s + 1]]
        sc = (4.0 ** cnt[dd, :]).sum(0)
        sc = sc + (szs >= cap) * 1e30
        c = int(sc.argmin())
        cls[s] = c
        szs[c] += 1
        cnt[dd, c] += 1

    # ---- node -> (core, position): cluster similar in-profiles per tile ----
    # (the greedy's cnt undercounts parallel edges due to fancy-index
    # buffering; recompute exactly for the layout sort)
    cnt = np.zeros((N, NCLS), np.int32)
    np.add.at(cnt, (dst, cls[src]), 1)
    blocks = []                                   # per core: array of node ids (-1 = dummy)
    for c in range(NCLS):
        nodes_c = np.flatnonzero(cls == c)
        cc = cnt[nodes_c]
        order = np.lexsort((cc[:, 3], cc[:, 2], cc[:, 1], cc[:, 0],
                            cc.argmax(1), cc.max(1)))
        nodes_c = nodes_c[order]
        a = np.full(CPN, -1, np.int64)
        b = np.full(CPN, -1, np.int64)
        a[: (len(nodes_c) + 1) // 2] = nodes_c[0::2]
        b[: len(nodes_c) // 2] = nodes_c[1::2]
        blocks.append(a)
        blocks.append(b)

    # table row of node n (within the AllGathered table)
    row = np.full(N, -1, np.int64)
    for k in range(NCORES):
        blk = blocks[k]
        real = blk >= 0
        row[blk[real]] = k * BLK + np.flatnonzero(real)

    # ---- per-(core,tile,class,partition) in-neighbor counts ----
    dcore = np.empty(N, np.int64)
    dpos = np.empty(N, np.int64)
    for k in range(NCORES):
        blk = blocks[k]
        real = blk >= 0
        dcore[blk[real]] = k
        dpos[blk[real]] = np.flatnonzero(real)
    ecore = dcore[dst]
    epos = dpos[dst]
    etile = epos // 128
    epart = epos % 128
    ecls = cls[src].astype(np.int64)
    esrow = row[src] - ecls * CLS_ROWS            # class-local table row (0..25089)
    assert esrow.min() >= 0 and esrow.max() < CLS_ROWS

    key = ((ecore * NT + etile) * NCLS + ecls) * 128 + epart
    eorder = np.argsort(key, kind="stable")
    key_s = key[eorder]
    esrow_s = esrow[eorder]
    counts = np.bincount(key_s, minlength=NCORES * NT * NCLS * 128)
    counts = counts.reshape(NCORES, NT, NCLS, 128)
    kmax = counts.max(axis=(0, 3))                # common K per (tile,class), max over cores
    kmax = np.maximum(kmax, 1)                    # avoid zero-length segments

    # ---- call grouping: consecutive tiles, per-class slot sum <= GB_SLOTS ----
    groups = []
    cur = []
    for t in range(NT):
        trial = cur + [t]
        if cur and max(kmax[trial, c].sum() for c in range(NCLS)) > GB_SLOTS:
            groups.append(cur)
            cur = [t]
        else:
            cur = trial
        if kmax[t].max() > GB_SLOTS:
            raise RuntimeError("single tile exceeds gather buffer")
    groups.append(cur)

    # calls: (class, tiles, seg_offsets, nslots); same order for both layers
    calls = []
    for g in groups:
        for c in range(NCLS):
            offs = np.concatenate([[0], np.cumsum(kmax[g, c])])
            calls.append((c, list(g), offs[:-1].tolist(), int(offs[-1])))

    total_slots = sum(nsl for (_, _, _, nsl) in calls)

    # ---- build per-core int16 index stream in call order (vectorized) ----
    flat_counts = counts.reshape(-1)
    starts = np.concatenate([[0], np.cumsum(flat_counts)])[:-1].reshape(NCORES, NT, NCLS, 128)
    epart_s = epart[eorder]

    streams = []
    for k in range(NCORES):
        stream = np.full(total_slots * 128, PAD_LOCAL, np.int16)
        base = 0
        for (c, tiles, offs, nsl) in calls:
            for t, off in zip(tiles, offs):
                cnts = counts[k, t, c]            # [128]
                st = starts[k, t, c]              # [128]
                tot = int(cnts.sum())
                if tot:
                    sl = slice(st[0], st[0] + tot)
                    parts = epart_s[sl]
                    jj = np.arange(tot) - np.repeat(st - st[0], cnts)
                    stream[base + (off + jj) * 128 + parts] = esrow_s[sl].astype(np.int16)
            base += nsl * 128
        # wrap for dma_gather: idxs[p, j] = stream[j*16 + p%16]
        wrapped = stream.reshape(-1, 16).T        # [16, cols]
        streams.append(np.tile(wrapped, (8, 1)))  # [128, cols]

    meta = dict(blocks=blocks, row=row, dinv=dinv, calls=calls, groups=groups,
                total_slots=total_slots, kmax=kmax)
    return meta, streams


def _build_program(calls, groups):
    import concourse.bacc as bacc
    import concourse.bass as bass
    from concourse import mybir
    from concourse.library_config import mlp
    from contextlib import ExitStack

    AF = mybir.ActivationFunctionType
    OP = mybir.AluOpType
    nc = bacc.Bacc("TRN2", target_bir_lowering=False, debug=False)

    NCALLS = len(calls)
    total_slots = sum(nsl for (_, _, _, nsl) in calls)
    COLS = total_slots * 8                        # int16 idx cols per partition
    # tile-aligned upload chunk boundaries (in cols)
    tchunk = [0, 25, 50, 75, NT1]
    xbound = [t * 128 for t in tchunk]

    xT = nc.declare_dram_parameter("xT", [128, XCOLS], mybir.dt.float32, isOutput=False)
    idxp = nc.declare_dram_parameter("idx", [128, COLS], mybir.dt.int16, isOutput=False)
    w1t = nc.declare_dram_parameter("w1t", [128, NHID], mybir.dt.float32, isOutput=False)
    w2p = nc.declare_dram_parameter("w2p", [NHID + 1, F], mybir.dt.float32, isOutput=False)
    b1b = nc.declare_dram_parameter("b1b", [128, 2 * NHID], mybir.dt.float32, isOutput=False)
    dvc = nc.declare_dram_parameter("dvc", [128, NT1], mybir.dt.float32, isOutput=False)
    idn = nc.declare_dram_parameter("idn", [128, 128], mybir.dt.float32, isOutput=False)
    zro = nc.declare_dram_parameter("zro", [1, F], mybir.dt.float32, isOutput=False)
    outp = nc.declare_dram_parameter("out", [CPN, NCLASS], mybir.dt.float32, isOutput=True)

    h1_own = nc.dram_tensor("h1_own", [XCOLS, F], mybir.dt.float32)
    h2_own = nc.dram_tensor("h2_own", [BLK, F], mybir.dt.float32)
    h1_full = nc.dram_tensor("h1_full", [TBL, F], mybir.dt.float32, addr_space="Shared")
    h2_full = nc.dram_tensor("h2_full", [TBL, F], mybir.dt.float32, addr_space="Shared")

    with ExitStack() as stack:
        ec = stack.enter_context
        block = ec(nc.Block())
        xT_sb = ec(nc.sbuf_tensor("xT_sb", [128, XCOLS], mybir.dt.float32))
        idx_sb = ec(nc.sbuf_tensor("idx_sb", [128, COLS], mybir.dt.int16))
        w1t_sb = ec(nc.sbuf_tensor("w1t_sb", [128, NHID], mybir.dt.float32))
        w2p_sb = ec(nc.sbuf_tensor("w2p_sb", [NHID + 1, F], mybir.dt.float32))
        b1b_sb = ec(nc.sbuf_tensor("b1b_sb", [128, 2 * NHID], mybir.dt.float32))
        dvc_sb = ec(nc.sbuf_tensor("dvc_sb", [128, NT1], mybir.dt.float32))
        idn_sb = ec(nc.sbuf_tensor("idn_sb", [128, 128], mybir.dt.float32))
        zro_sb = ec(nc.sbuf_tensor("zro_sb", [1, F], mybir.dt.float32))
        gbuf = ec(nc.sbuf_tensor("gbuf", [128, GB_BUFS, GB_SLOTS, F], mybir.dt.float32))
        agg = ec(nc.sbuf_tensor("agg", [128, NT, F], mybir.dt.float32))
        hs = ec(nc.sbuf_tensor("hs", [128, NT1, F], mybir.dt.float32))
        x2T = ec(nc.sbuf_tensor("x2T", [NHID + 1, 2, 128], mybir.dt.float32))
        osb = ec(nc.sbuf_tensor("osb", [128, ORING, NCLASS], mybir.dt.float32))
        tmp = ec(nc.sbuf_tensor("tmp", [128, F], mybir.dt.float32))
        tmp2 = ec(nc.sbuf_tensor("tmp2", [128, NCLASS], mybir.dt.float32))
        lse = ec(nc.sbuf_tensor("lse", [128, NT], mybir.dt.float32))
        lnl = ec(nc.sbuf_tensor("lnl", [128, NT], mybir.dt.float32))
        ph1 = ec(nc.psum_tensor("ph1", [128, 4, NHID], mybir.dt.float32))
        pT = ec(nc.psum_tensor("pT", [NHID, 2, 512], mybir.dt.float32))
        p2 = ec(nc.psum_tensor("p2", [128, 2, 512], mybir.dt.float32))
        sems = {n: ec(nc.semaphore(n)) for n in [
            "s_in", "s_xin", "s_idx", "s_mm1", "s_ep1", "s_st1", "s_cc", "s_g",
            "s_red", "s_x2", "s_tp", "s_cp", "s_mm2", "s_ep2", "s_st2", "s_z",
            "s_ln", "s_sm", "s_out"]}
        (s_in, s_xin, s_idx, s_mm1, s_ep1, s_st1, s_cc, s_g, s_red, s_x2, s_tp,
         s_cp, s_mm2, s_ep2, s_st2, s_z, s_ln, s_sm, s_out) = (
            sems[n] for n in ["s_in", "s_xin", "s_idx", "s_mm1", "s_ep1",
                              "s_st1", "s_cc", "s_g", "s_red", "s_x2", "s_tp",
                              "s_cp", "s_mm2", "s_ep2", "s_st2", "s_z", "s_ln",
                              "s_sm", "s_out"])

        # ---------------- sync engine: uploads + stores ----------------
        @block.sync
        def _(se: bass.BassEngine):
            se.dma_start(w1t_sb[:], w1t[:]).then_inc(s_in, 16)       # s_in 16
            se.dma_start(b1b_sb[:], b1b[:]).then_inc(s_in, 16)       # s_in 32
            se.dma_start(dvc_sb[:], dvc[:]).then_inc(s_in, 16)       # s_in 48
            for c in range(NXCH):
                se.dma_start(xT_sb[:, xbound[c]:xbound[c + 1]],
                             xT[:, xbound[c]:xbound[c + 1]]).then_inc(s_xin, 16)
            se.dma_start(idn_sb[:], idn[:]).then_inc(s_in, 16)       # s_in 64
            se.dma_start(w2p_sb[:], w2p[:]).then_inc(s_in, 16)       # s_in 80
            se.dma_start(zro_sb[:], zro[:]).then_inc(s_in, 16)       # s_in 96
            # h2 zero pad row (zro_sb upload must have landed first)
            se.wait_ge(s_in, 96)
            se.dma_start(h2_own[CPN:CPN + 1, :], zro_sb[:]).then_inc(s_st2, 16)
            # phase-1 stores, 3 tiles per DMA (99 = 33*3)
            for i in range(NT1 // 3):
                t = 3 * i
                se.wait_ge(s_ep1, t + 3)
                dst_ap = h1_own[t * 128:(t + 3) * 128, :].rearrange("(k p) f -> p k f", p=128)
                se.dma_start(dst_ap, hs[:, t:t + 3, :]).then_inc(s_st1, 16)
            # layer-2 h2' stores, 2 tiles per DMA (98 = 49*2)
            for i in range(NT // 2):
                t = 2 * i
                se.wait_ge(s_ep2, t + 2)
                dst_ap = h2_own[t * 128:(t + 2) * 128, :].rearrange("(k p) f -> p k f", p=128)
                se.dma_start(dst_ap, hs[:, t:t + 2, :]).then_inc(s_st2, 16)
            # output stores, 2 tiles per DMA
            for i in range(NT // 2):
                t = 2 * i
                se.wait_ge(s_sm, t + 2)
                dst_ap = outp[t * 128:(t + 2) * 128, :].rearrange("(k p) f -> p k f", p=128)
                se.dma_start(dst_ap, osb[:, t % ORING:t % ORING + 2, :]).then_inc(s_out, 16)
            se.wait_ge(s_out, 16 * (NT // 2))

        # ---------------- gpsimd: idx upload, collectives, gathers ----------------
        @block.gpsimd
        def _(g: bass.BassGpSimd):
            g.load_library(mlp)
            g.dma_start(idx_sb[:], idxp[:]).then_inc(s_idx, 16)
            g.wait_ge(s_idx, 16)
            for layer in (0, 1):
                tblt = h1_full if layer == 0 else h2_full
                if layer == 0:
                    g.wait_ge(s_st1, 16 * (NT1 // 3))
                    g.collective_compute(
                        "AllGather", mybir.AluOpType.bypass,
                        replica_groups=[list(range(NCORES))],
                        ins=[h1_own[0:BLK, :].opt()],
                        outs=[h1_full[:, :].opt()],
                    ).then_inc(s_cc)
                    g.wait_ge(s_cc, 1)
                else:
                    g.wait_ge(s_st2, 16 * (NT // 2 + 1))
                    g.collective_compute(
                        "AllGather", mybir.AluOpType.bypass,
                        replica_groups=[list(range(NCORES))],
                        ins=[h2_own[:, :].opt()],
                        outs=[h2_full[:, :].opt()],
                    ).then_inc(s_cc)
                    g.wait_ge(s_cc, 2)
                off = 0
                for j, (c, tiles, offs, nsl) in enumerate(calls):
                    gj = layer * NCALLS + j
                    if gj >= GB_BUFS:
                        g.wait_ge(s_red, gj - GB_BUFS + 1)
                    nidx = nsl * 128
                    g.dma_gather(
                        gbuf[:, gj % GB_BUFS, :nsl, :],
                        tblt[c * CLS_ROWS:(c + 1) * CLS_ROWS, :],
                        idx_sb[:, off * 8:(off + nsl) * 8],
                        nidx, nidx, F,
                        single_packet=False,
                    ).then_inc(s_g, 16)
                    off += nsl
                off = 0

        # ---------------- tensor engine ----------------
        @block.tensor
        def _(te):
            te.wait_ge(s_in, 16)
            for t in range(NT1):
                te.wait_ge(s_xin, 16 * (t // 25 + 1))
                if t >= 4:
                    te.wait_ge(s_ep1, t - 3)
                te.matmul(ph1[:, t % 4, :], xT_sb[:, t * 128:(t + 1) * 128], w1t_sb[:]).then_inc(s_mm1)
            # layer-2: software-pipelined transpose / matmul2
            te.wait_ge(s_in, 80)
            for t in range(NT):
                te.wait_ge(s_x2, t + 1)
                if t >= 2:
                    te.wait_ge(s_cp, t - 1)
                te.transpose(pT[:, t % 2, :128], agg[:, t, :], idn_sb[:]).then_inc(s_tp)
                if t >= 1:
                    te.wait_ge(s_cp, t)
                    if t >= 3:
                        te.wait_ge(s_ep2, t - 2)
                    te.matmul(p2[:, (t - 1) % 2, :F], x2T[:, (t - 1) % 2, :], w2p_sb[:]).then_inc(s_mm2)
            te.wait_ge(s_cp, NT)
            te.wait_ge(s_ep2, NT - 2)
            te.matmul(p2[:, (NT - 1) % 2, :F], x2T[:, (NT - 1) % 2, :], w2p_sb[:]).then_inc(s_mm2)

        # ---------------- vector engine ----------------
        @block.vector
        def _(v: bass.BassVectorEngine):
            v.wait_ge(s_in, 48)
            v.memset(x2T[NHID:NHID + 1, :, :], 1.0)   # bias row for matmul2
            # phase 1: h1' tiles (persistent); bias-add batched over tile pairs
            for t in range(0, NT1 - 1, 2):
                v.wait_ge(s_mm1, t + 2)
                v.tensor_add(hs[:, t:t + 2, :], ph1[:, t % 4:t % 4 + 2, :],
                             b1b_sb[:].rearrange("p (k f) -> p k f", k=2))
                v.tensor_scalar(out=hs[:, t, :], in0=hs[:, t, :],
                                scalar1=dvc_sb[:, t:t + 1], scalar2=None,
                                op0=OP.mult).then_inc(s_ep1)
                v.tensor_scalar(out=hs[:, t + 1, :], in0=hs[:, t + 1, :],
                                scalar1=dvc_sb[:, t + 1:t + 2], scalar2=None,
                                op0=OP.mult).then_inc(s_ep1)
            t = NT1 - 1
            v.wait_ge(s_mm1, t + 1)
            v.tensor_add(hs[:, t, :], ph1[:, t % 4, :], b1b_sb[:, :NHID])
            v.tensor_scalar(out=hs[:, t, :], in0=hs[:, t, :],
                            scalar1=dvc_sb[:, t:t + 1], scalar2=None,
                            op0=OP.mult).then_inc(s_ep1)
            # self-loop agg init (runs under AllGather-1)
            for t in range(NT):
                v.tensor_scalar(out=agg[:, t, :], in0=hs[:, t, :],
                                scalar1=dvc_sb[:, t:t + 1], scalar2=None,
                                op0=OP.mult)

            def _final(gi):
                for t in groups[gi]:
                    v.wait_ge(s_ln, gi + 1)
                    if t >= ORING:
                        v.wait_ge(s_out, 16 * ((t - ORING) // 2 + 1))
                    v.tensor_scalar(out=osb[:, t % ORING, :], in0=agg[:, t, :NCLASS],
                                    scalar1=dvc_sb[:, t:t + 1], scalar2=lnl[:, t:t + 1],
                                    op0=OP.mult, op1=OP.subtract).then_inc(s_sm)

            def _ep2(t):
                # h2' tile + layer-2 self-loop agg init (overwrites x2 in agg)
                v.wait_ge(s_mm2, t + 1)
                v.tensor_scalar(out=hs[:, t, :], in0=p2[:, t % 2, :F],
                                scalar1=dvc_sb[:, t:t + 1], scalar2=None,
                                op0=OP.mult).then_inc(s_ep2)
                v.tensor_scalar(out=agg[:, t, :], in0=p2[:, t % 2, :F],
                                scalar1=dvc_sb[:, t:t + 1], scalar2=dvc_sb[:, t:t + 1],
                                op0=OP.mult, op1=OP.mult)

            for layer in (0, 1):
                for gi, gtiles in enumerate(groups):
                    for c in range(NCLS):
                        j = gi * NCLS + c
                        (_, tiles, offs, nsl) = calls[j]
                        gj = layer * NCALLS + j
                        v.wait_ge(s_g, 16 * (gj + 1))
                        for ti, t in enumerate(tiles):
                            off = offs[ti]
                            K = (offs[ti + 1] - offs[ti]) if ti + 1 < len(tiles) else nsl - offs[ti]
                            seg = gbuf[:, gj % GB_BUFS, off:off + K, :].rearrange("p k f -> p f k")
                            v.tensor_reduce(tmp[:], seg, axis=mybir.AxisListType.X, op=OP.add)
                            ta = v.tensor_add(agg[:, t, :], agg[:, t, :], tmp[:])
                            if layer == 1 and c == NCLS - 1:
                                ta.then_inc(s_z)
                        v.nop().then_inc(s_red, 1)
                    if layer == 0:
                        # x2 = relu(dinv * agg), then pipelined transpose-copy/ep2
                        for t in gtiles:
                            v.tensor_scalar(out=agg[:, t, :], in0=agg[:, t, :],
                                            scalar1=dvc_sb[:, t:t + 1], scalar2=0.0,
                                            op0=OP.mult, op1=OP.max).then_inc(s_x2)
                        for t in gtiles:
                            v.wait_ge(s_tp, t + 1)
                            v.tensor_copy(x2T[:NHID, t % 2, :], pT[:, t % 2, :128]).then_inc(s_cp)
                            if t >= 1:
                                _ep2(t - 1)
                    else:
                        # final: out = dinv*agg - ln(sum exp), one group behind
                        # so the Act exp/Ln round trip hides under gathers
                        if gi >= 1:
                            _final(gi - 1)
                if layer == 0:
                    _ep2(NT - 1)
                else:
                    _final(len(groups) - 1)

        # ---------------- scalar engine: exp accumulate + per-group Ln ----------------
        @block.scalar
        def _(sc):
            sc.wait_ge(s_in, 48)
            for gi, gtiles in enumerate(groups):
                for t in gtiles:
                    sc.wait_ge(s_z, t + 1)
                    sc.activation(tmp2[:], agg[:, t, :NCLASS], AF.Exp,
                                  scale=dvc_sb[:, t:t + 1],
                                  accum_out=lse[:, t:t + 1])
                t0, t1 = gtiles[0], gtiles[-1] + 1
                sc.activation(lnl[:, t0:t1], lse[:, t0:t1], AF.Ln).then_inc(s_ln)

    nc.compile()
    return nc


_LAST_NC = None


def kernel(x, W1, b1, W2, b2, edge_index):
    global _LAST_NC
    from concourse.bass_utils import run_bass_kernel_spmd

    x = np.asarray(x)
    W1 = np.asarray(W1); b1 = np.asarray(b1)
    W2 = np.asarray(W2); b2 = np.asarray(b2)
    edge_index = np.asarray(edge_index)

    meta, streams = _host_prep(edge_index)
    calls = meta["calls"]
    groups = meta["groups"]
    nc = _build_program(calls, groups)
    _LAST_NC = nc

    dinv = meta["dinv"]
    blocks = meta["blocks"]
    ident = np.eye(128, dtype=np.float32)
    w1t_np = W1.T.astype(np.float32).copy()                      # [128,64]
    w2p_np = np.zeros((NHID + 1, F), np.float32)
    w2p_np[:NHID, :NCLASS] = W2.T
    w2p_np[NHID, :NCLASS] = b2                                   # bias row
    b1b_np = np.tile(b1.astype(np.float32), (128, 2))

    in_maps = []
    for k in range(NCORES):
        blk = blocks[k]
        real = blk >= 0
        xTk = np.zeros((128, XCOLS), np.float32)
        dvk = np.zeros(XCOLS, np.float32)
        idxs = np.flatnonzero(real)
        xcols = np.zeros((XCOLS, NFEAT), np.float32)
        xcols[idxs] = x[blk[idxs]]
        xTk[:, :] = xcols.T
        dvk[idxs] = dinv[blk[idxs]]
        dvc_np = dvk.reshape(NT1, 128).T.copy()                  # [128, NT1]
        in_maps.append({
            "xT": xTk, "idx": streams[k], "w1t": w1t_np, "w2p": w2p_np,
            "b1b": b1b_np, "dvc": dvc_np, "idn": ident,
            "zro": np.zeros((1, F), np.float32),
        })

    res = run_bass_kernel_spmd(nc, in_maps, list(range(NCORES)))

    out = np.empty((N, NCLASS), np.float32)
    for k in range(NCORES):
        blk = blocks[k]
        real = blk >= 0
        out[blk[real]] = res.results[k]["out"][np.flatnonzero(real)]
    return out


# revision 30
# speedup vs baseline: 1.0634x; 1.0084x over previous
"""2-layer GCN (gnn_message_passing) on 8 Trainium2 NeuronCores.

Strategy (v2):
  - Fold the symmetric degree normalization into per-node scaling:
      msg_e = dinv[src]*dinv[dst]*h[src]  =>  agg = dinv * A_sum(dinv * h)
    so aggregation is a pure unweighted gather + segment-sum.
  - Node-partition the graph over 8 cores (12544 dst nodes per core).
  - Each core computes h' = (x_shard @ W^T + b) * dinv for its shard,
    AllGathers the full h' table (f32 rows of 64 floats = 256B), then
    aggregates its own dst tiles with batched dma_gather (int16 indices
    -> 4 table "classes" of <=25090 rows each) + strided vector reduces.
  - Self-loop terms never enter the gather streams: each dst tile's
    accumulator is initialized with dinv^2 * h from the SBUF-resident
    h' tiles.
  - Dst nodes are clustered into tiles by their (max, argmax, full
    vector) in-neighbor class profile so the per-(tile,class) padded
    slot count tracks the actual in-degree profile (~22% padding).
  - Layer-1 epilogue (relu fused into the dinv scale, transpose,
    matmul2 with bias folded in via a ones row) is pipelined per tile
    group under the remaining gathers, so AllGather-2 launches right
    after the last layer-1 reduce. The layer-2 softmax pipeline (exp
    with accumulate, one deferred Ln over all tiles, fused
    subtract+store) runs under the layer-2 gathers.
"""

import sys
import numpy as np

sys.path.insert(0, "/opt/trn_rl_repo")

N = 100000
E = 1600000
NFEAT, NHID, NCLASS = 128, 64, 40
NCORES = 8
CPN = 12544            # dst nodes per core (98 tiles of 128)
BLK = CPN + 1          # AllGather block rows per core (+1 zero pad row)
NT = CPN // 128        # 98 aggregation tiles per core
NT1 = NT + 1           # 99 phase-1 tiles (covers the pad row)
XCOLS = NT1 * 128      # 12672
NCLS = 4
CLS_ROWS = 2 * BLK     # 25090 table rows per class (= 2 core blocks)
TBL = NCORES * BLK     # 100360
PAD_LOCAL = CPN        # local index of the zero row within a class
GB_SLOTS = 32          # gather ring buffer slots per call
GB_BUFS = 8
ORING = 4              # output store ring
F = 64                 # table row width (f32) = 256B
NXCH = 4               # xT upload chunks


def _host_prep(edge_index):
    """Returns permutation/class layout + per-core padded gather streams."""
    src = edge_index[0].astype(np.int64)
    dst = edge_index[1].astype(np.int64)
    deg = np.bincount(src, minlength=N) + 1      # +1: self-loop
    dinv = (1.0 / np.sqrt(deg.astype(np.float64))).astype(np.float32)

    # ---- greedy class assignment of sources (balance each dst's in-nbrs) ----
    order_e = np.argsort(src, kind="stable")
    d_sorted = dst[order_e]
    sptr = np.searchsorted(src[order_e], np.arange(N + 1))
    cap = NCORES * CPN // NCLS                    # 25088 real nodes max per class
    cnt = np.zeros((N, NCLS), np.int32)
    cls = np.full(N, -1, np.int8)
    szs = np.zeros(NCLS, np.int64)
    outdeg = np.bincount(src, minlength=N)
    sorder = np.argsort(-outdeg, kind="stable")
    for s in sorder:
        dd = d_sorted[sptr[s]:sptr[s + 1]]
        sc = (4.0 ** cnt[dd, :]).sum(0)
        sc = sc + (szs >= cap) * 1e30
        c = int(sc.argmin())
        cls[s] = c
        szs[c] += 1
        cnt[dd, c] += 1

    # ---- node -> (core, position): cluster similar in-profiles per tile ----
    # (the greedy's cnt undercounts parallel edges due to fancy-index
    # buffering; recompute exactly for the layout sort)
    cnt = np.zeros((N, NCLS), np.int32)
    np.add.at(cnt, (dst, cls[src]), 1)
    blocks = []                                   # per core: array of node ids (-1 = dummy)
    for c in range(NCLS):
        nodes_c = np.flatnonzero(cls == c)
        cc = cnt[nodes_c]
        order = np.lexsort((cc[:, 3], cc[:, 2], cc[:, 1], cc[:, 0],
                            cc.argmax(1), cc.max(1)))
        nodes_c = nodes_c[order]
        a = np.full(CPN, -1, np.int64)
        b = np.full(CPN, -1, np.int64)
        a[: (len(nodes_c) + 1) // 2] = nodes_c[0::2]
        b[: len(nodes_c) // 2] = nodes_c[1::2]
        blocks.append(a)
        blocks.append(b)

    # table row of node n (within the AllGathered table)
    row = np.full(N, -1, np.int64)
    for k in range(NCORES):
        blk = blocks[k]
        real = blk >= 0
        row[blk[real]] = k * BLK + np.flatnonzero(real)

    # ---- per-(core,tile,class,partition) in-neighbor counts ----
    dcore = np.empty(N, np.int64)
    dpos = np.empty(N, np.int64)
    for k in range(NCORES):
        blk = blocks[k]
        real = blk >= 0
        dcore[blk[real]] = k
        dpos[blk[real]] = np.flatnonzero(real)
    ecore = dcore[dst]
    epos = dpos[dst]
    etile = epos // 128
    epart = epos % 128
    ecls = cls[src].astype(np.int64)
    esrow = row[src] - ecls * CLS_ROWS            # class-local table row (0..25089)
    assert esrow.min() >= 0 and esrow.max() < CLS_ROWS

    key = ((ecore * NT + etile) * NCLS + ecls) * 128 + epart
    eorder = np.argsort(key, kind="stable")
    key_s = key[eorder]
    esrow_s = esrow[eorder]
    counts = np.bincount(key_s, minlength=NCORES * NT * NCLS * 128)
    counts = counts.reshape(NCORES, NT, NCLS, 128)
    kmax = counts.max(axis=(0, 3))                # common K per (tile,class), max over cores
    kmax = np.maximum(kmax, 1)                    # avoid zero-length segments

    # ---- call grouping: consecutive tiles, per-class slot sum <= GB_SLOTS ----
    groups = []
    cur = []
    for t in range(NT):
        trial = cur + [t]
        if cur and max(kmax[trial, c].sum() for c in range(NCLS)) > GB_SLOTS:
            groups.append(cur)
            cur = [t]
        else:
            cur = trial
        if kmax[t].max() > GB_SLOTS:
            raise RuntimeError("single tile exceeds gather buffer")
    groups.append(cur)

    # calls: (class, tiles, seg_offsets, nslots); same order for both layers
    calls = []
    for g in groups:
        for c in range(NCLS):
            offs = np.concatenate([[0], np.cumsum(kmax[g, c])])
            calls.append((c, list(g), offs[:-1].tolist(), int(offs[-1])))

    total_slots = sum(nsl for (_, _, _, nsl) in calls)

    # ---- build per-core int16 index stream in call order (vectorized) ----
    flat_counts = counts.reshape(-1)
    starts = np.concatenate([[0], np.cumsum(flat_counts)])[:-1].reshape(NCORES, NT, NCLS, 128)
    epart_s = epart[eorder]

    streams = []
    for k in range(NCORES):
        stream = np.full(total_slots * 128, PAD_LOCAL, np.int16)
        base = 0
        for (c, tiles, offs, nsl) in calls:
            for t, off in zip(tiles, offs):
                cnts = counts[k, t, c]            # [128]
                st = starts[k, t, c]              # [128]
                tot = int(cnts.sum())
                if tot:
                    sl = slice(st[0], st[0] + tot)
                    parts = epart_s[sl]
                    jj = np.arange(tot) - np.repeat(st - st[0], cnts)
                    stream[base + (off + jj) * 128 + parts] = esrow_s[sl].astype(np.int16)
            base += nsl * 128
        # wrap for dma_gather: idxs[p, j] = stream[j*16 + p%16]
        wrapped = stream.reshape(-1, 16).T        # [16, cols]
        streams.append(np.tile(wrapped, (8, 1)))  # [128, cols]

    meta = dict(blocks=blocks, row=row, dinv=dinv, calls=calls, groups=groups,
                total_slots=total_slots, kmax=kmax)
    return meta, streams


def _build_program(calls, groups):
    import concourse.bacc as bacc
    import concourse.bass as bass
    from concourse import mybir
    from concourse.library_config import mlp
    from contextlib import ExitStack

    AF = mybir.ActivationFunctionType
    OP = mybir.AluOpType
    nc = bacc.Bacc("TRN2", target_bir_lowering=False, debug=False)

    NCALLS = len(calls)
    total_slots = sum(nsl for (_, _, _, nsl) in calls)
    COLS = total_slots * 8                        # int16 idx cols per partition
    # tile-aligned upload chunk boundaries (in cols)
    tchunk = [0, 25, 50, 75, NT1]
    xbound = [t * 128 for t in tchunk]

    xT = nc.declare_dram_parameter("xT", [128, XCOLS], mybir.dt.float32, isOutput=False)
    idxp = nc.declare_dram_parameter("idx", [128, COLS], mybir.dt.int16, isOutput=False)
    w1t = nc.declare_dram_parameter("w1t", [128, NHID], mybir.dt.float32, isOutput=False)
    w2p = nc.declare_dram_parameter("w2p", [NHID + 1, F], mybir.dt.float32, isOutput=False)
    b1b = nc.declare_dram_parameter("b1b", [128, 4 * NHID], mybir.dt.float32, isOutput=False)
    dvc = nc.declare_dram_parameter("dvc", [128, NT1], mybir.dt.float32, isOutput=False)
    idn = nc.declare_dram_parameter("idn", [128, 128], mybir.dt.float32, isOutput=False)
    zro = nc.declare_dram_parameter("zro", [1, F], mybir.dt.float32, isOutput=False)
    outp = nc.declare_dram_parameter("out", [CPN, NCLASS], mybir.dt.float32, isOutput=True)

    h1_own = nc.dram_tensor("h1_own", [XCOLS, F], mybir.dt.float32)
    h2_own = nc.dram_tensor("h2_own", [BLK, F], mybir.dt.float32)
    h1_full = nc.dram_tensor("h1_full", [TBL, F], mybir.dt.float32, addr_space="Shared")
    h2_full = nc.dram_tensor("h2_full", [TBL, F], mybir.dt.float32, addr_space="Shared")

    with ExitStack() as stack:
        ec = stack.enter_context
        block = ec(nc.Block())
        xT_sb = ec(nc.sbuf_tensor("xT_sb", [128, XCOLS], mybir.dt.float32))
        idx_sb = ec(nc.sbuf_tensor("idx_sb", [128, COLS], mybir.dt.int16))
        w1t_sb = ec(nc.sbuf_tensor("w1t_sb", [128, NHID], mybir.dt.float32))
        w2p_sb = ec(nc.sbuf_tensor("w2p_sb", [NHID + 1, F], mybir.dt.float32))
        b1b_sb = ec(nc.sbuf_tensor("b1b_sb", [128, 4 * NHID], mybir.dt.float32))
        dvc_sb = ec(nc.sbuf_tensor("dvc_sb", [128, NT1], mybir.dt.float32))
        idn_sb = ec(nc.sbuf_tensor("idn_sb", [128, 128], mybir.dt.float32))
        zro_sb = ec(nc.sbuf_tensor("zro_sb", [1, F], mybir.dt.float32))
        gbuf = ec(nc.sbuf_tensor("gbuf", [128, GB_BUFS, GB_SLOTS, F], mybir.dt.float32))
        agg = ec(nc.sbuf_tensor("agg", [128, NT, F], mybir.dt.float32))
        hs = ec(nc.sbuf_tensor("hs", [128, NT1, F], mybir.dt.float32))
        x2T = ec(nc.sbuf_tensor("x2T", [NHID + 1, 2, 128], mybir.dt.float32))
        osb = ec(nc.sbuf_tensor("osb", [128, ORING, NCLASS], mybir.dt.float32))
        tmp = ec(nc.sbuf_tensor("tmp", [128, F], mybir.dt.float32))
        tmp2 = ec(nc.sbuf_tensor("tmp2", [128, NCLASS], mybir.dt.float32))
        lse = ec(nc.sbuf_tensor("lse", [128, NT], mybir.dt.float32))
        lnl = ec(nc.sbuf_tensor("lnl", [128, NT], mybir.dt.float32))
        ph1 = ec(nc.psum_tensor("ph1", [128, 8, NHID], mybir.dt.float32))
        pT = ec(nc.psum_tensor("pT", [NHID, 2, 512], mybir.dt.float32))
        p2 = ec(nc.psum_tensor("p2", [128, 2, 512], mybir.dt.float32))
        sems = {n: ec(nc.semaphore(n)) for n in [
            "s_in", "s_xin", "s_idx", "s_mm1", "s_ep1", "s_st1", "s_cc", "s_g",
            "s_red", "s_x2", "s_tp", "s_cp", "s_mm2", "s_ep2", "s_st2", "s_z",
            "s_ln", "s_sm", "s_out"]}
        (s_in, s_xin, s_idx, s_mm1, s_ep1, s_st1, s_cc, s_g, s_red, s_x2, s_tp,
         s_cp, s_mm2, s_ep2, s_st2, s_z, s_ln, s_sm, s_out) = (
            sems[n] for n in ["s_in", "s_xin", "s_idx", "s_mm1", "s_ep1",
                              "s_st1", "s_cc", "s_g", "s_red", "s_x2", "s_tp",
                              "s_cp", "s_mm2", "s_ep2", "s_st2", "s_z", "s_ln",
                              "s_sm", "s_out"])

        # ---------------- sync engine: uploads + stores ----------------
        @block.sync
        def _(se: bass.BassEngine):
            se.dma_start(w1t_sb[:], w1t[:]).then_inc(s_in, 16)       # s_in 16
            se.dma_start(b1b_sb[:], b1b[:]).then_inc(s_in, 16)       # s_in 32
            se.dma_start(dvc_sb[:], dvc[:]).then_inc(s_in, 16)       # s_in 48
            for c in range(NXCH):
                se.dma_start(xT_sb[:, xbound[c]:xbound[c + 1]],
                             xT[:, xbound[c]:xbound[c + 1]]).then_inc(s_xin, 16)
            se.dma_start(idn_sb[:], idn[:]).then_inc(s_in, 16)       # s_in 64
            se.dma_start(w2p_sb[:], w2p[:]).then_inc(s_in, 16)       # s_in 80
            se.dma_start(zro_sb[:], zro[:]).then_inc(s_in, 16)       # s_in 96
            # h2 zero pad row (zro_sb upload must have landed first)
            se.wait_ge(s_in, 96)
            se.dma_start(h2_own[CPN:CPN + 1, :], zro_sb[:]).then_inc(s_st2, 16)
            # phase-1 stores, 3 tiles per DMA (99 = 33*3)
            for i in range(NT1 // 3):
                t = 3 * i
                se.wait_ge(s_ep1, t + 3)
                dst_ap = h1_own[t * 128:(t + 3) * 128, :].rearrange("(k p) f -> p k f", p=128)
                se.dma_start(dst_ap, hs[:, t:t + 3, :]).then_inc(s_st1, 16)
            # layer-2 h2' stores, 2 tiles per DMA (98 = 49*2)
            for i in range(NT // 2):
                t = 2 * i
                se.wait_ge(s_ep2, t + 2)
                dst_ap = h2_own[t * 128:(t + 2) * 128, :].rearrange("(k p) f -> p k f", p=128)
                se.dma_start(dst_ap, hs[:, t:t + 2, :]).then_inc(s_st2, 16)
            # output stores, 2 tiles per DMA
            for i in range(NT // 2):
                t = 2 * i
                se.wait_ge(s_sm, t + 2)
                dst_ap = outp[t * 128:(t + 2) * 128, :].rearrange("(k p) f -> p k f", p=128)
                se.dma_start(dst_ap, osb[:, t % ORING:t % ORING + 2, :]).then_inc(s_out, 16)
            se.wait_ge(s_out, 16 * (NT // 2))

        # ---------------- gpsimd: idx upload, collectives, gathers ----------------
        @block.gpsimd
        def _(g: bass.BassGpSimd):
            g.load_library(mlp)
            g.dma_start(idx_sb[:], idxp[:]).then_inc(s_idx, 16)
            g.wait_ge(s_idx, 16)
            for layer in (0, 1):
                tblt = h1_full if layer == 0 else h2_full
                if layer == 0:
                    g.wait_ge(s_st1, 16 * (NT1 // 3))
                    g.collective_compute(
                        "AllGather", mybir.AluOpType.bypass,
                        replica_groups=[list(range(NCORES))],
                        ins=[h1_own[0:BLK, :].opt()],
                        outs=[h1_full[:, :].opt()],
                    ).then_inc(s_cc)
                    g.wait_ge(s_cc, 1)
                else:
                    g.wait_ge(s_st2, 16 * (NT // 2 + 1))
                    g.collective_compute(
                        "AllGather", mybir.AluOpType.bypass,
                        replica_groups=[list(range(NCORES))],
                        ins=[h2_own[:, :].opt()],
                        outs=[h2_full[:, :].opt()],
                    ).then_inc(s_cc)
                    g.wait_ge(s_cc, 2)
                off = 0
                for j, (c, tiles, offs, nsl) in enumerate(calls):
                    gj = layer * NCALLS + j
                    if gj >= GB_BUFS:
                        g.wait_ge(s_red, gj - GB_BUFS + 1)
                    nidx = nsl * 128
                    g.dma_gather(
                        gbuf[:, gj % GB_BUFS, :nsl, :],
                        tblt[c * CLS_ROWS:(c + 1) * CLS_ROWS, :],
                        idx_sb[:, off * 8:(off + nsl) * 8],
                        nidx, nidx, F,
                        single_packet=False,
                    ).then_inc(s_g, 16)
                    off += nsl
                off = 0

        # ---------------- tensor engine ----------------
        @block.tensor
        def _(te):
            te.wait_ge(s_in, 16)
            for t in range(NT1):
                te.wait_ge(s_xin, 16 * (t // 25 + 1))
                if t >= 8:
                    te.wait_ge(s_ep1, t - 7)
                te.matmul(ph1[:, t % 8, :], xT_sb[:, t * 128:(t + 1) * 128], w1t_sb[:]).then_inc(s_mm1)
            # layer-2: software-pipelined transpose / matmul2
            te.wait_ge(s_in, 80)
            for t in range(NT):
                te.wait_ge(s_x2, t + 1)
                if t >= 2:
                    te.wait_ge(s_cp, t - 1)
                te.transpose(pT[:, t % 2, :128], agg[:, t, :], idn_sb[:]).then_inc(s_tp)
                if t >= 1:
                    te.wait_ge(s_cp, t)
                    if t >= 3:
                        te.wait_ge(s_ep2, t - 2)
                    te.matmul(p2[:, (t - 1) % 2, :F], x2T[:, (t - 1) % 2, :], w2p_sb[:]).then_inc(s_mm2)
            te.wait_ge(s_cp, NT)
            te.wait_ge(s_ep2, NT - 2)
            te.matmul(p2[:, (NT - 1) % 2, :F], x2T[:, (NT - 1) % 2, :], w2p_sb[:]).then_inc(s_mm2)

        # ---------------- vector engine ----------------
        @block.vector
        def _(v: bass.BassVectorEngine):
            v.wait_ge(s_in, 48)
            v.memset(x2T[NHID:NHID + 1, :, :], 1.0)   # bias row for matmul2
            # phase 1: h1' tiles (persistent); bias-add batched over 4 tiles
            b1b4 = b1b_sb[:].rearrange("p (k f) -> p k f", k=4)
            for t0 in range(0, NT1, 4):
                nb = min(4, NT1 - t0)
                v.wait_ge(s_mm1, t0 + nb)
                if nb == 4:
                    v.tensor_add(hs[:, t0:t0 + 4, :], ph1[:, t0 % 8:t0 % 8 + 4, :], b1b4)
                else:
                    for t in range(t0, t0 + nb):
                        v.tensor_add(hs[:, t, :], ph1[:, t % 8, :], b1b_sb[:, :NHID])
                for t in range(t0, t0 + nb):
                    v.tensor_scalar(out=hs[:, t, :], in0=hs[:, t, :],
                                    scalar1=dvc_sb[:, t:t + 1], scalar2=None,
                                    op0=OP.mult).then_inc(s_ep1)
            # self-loop agg init (runs under AllGather-1)
            for t in range(NT):
                v.tensor_scalar(out=agg[:, t, :], in0=hs[:, t, :],
                                scalar1=dvc_sb[:, t:t + 1], scalar2=None,
                                op0=OP.mult)

            def _final(gi):
                for t in groups[gi]:
                    v.wait_ge(s_ln, gi + 1)
                    if t >= ORING:
                        v.wait_ge(s_out, 16 * ((t - ORING) // 2 + 1))
                    v.tensor_scalar(out=osb[:, t % ORING, :], in0=agg[:, t, :NCLASS],
                                    scalar1=dvc_sb[:, t:t + 1], scalar2=lnl[:, t:t + 1],
                                    op0=OP.mult, op1=OP.subtract).then_inc(s_sm)

            def _ep2(t):
                # h2' tile + layer-2 self-loop agg init (overwrites x2 in agg)
                v.wait_ge(s_mm2, t + 1)
                v.tensor_scalar(out=hs[:, t, :], in0=p2[:, t % 2, :F],
                                scalar1=dvc_sb[:, t:t + 1], scalar2=None,
                                op0=OP.mult).then_inc(s_ep2)
                v.tensor_scalar(out=agg[:, t, :], in0=p2[:, t % 2, :F],
                                scalar1=dvc_sb[:, t:t + 1], scalar2=dvc_sb[:, t:t + 1],
                                op0=OP.mult, op1=OP.mult)

            for layer in (0, 1):
                for gi, gtiles in enumerate(groups):
                    for c in range(NCLS):
                        j = gi * NCLS + c
                        (_, tiles, offs, nsl) = calls[j]
                        gj = layer * NCALLS + j
                        v.wait_ge(s_g, 16 * (gj + 1))
                        for ti, t in enumerate(tiles):
                            off = offs[ti]
                            K = (offs[ti + 1] - offs[ti]) if ti + 1 < len(tiles) else nsl - offs[ti]
                            seg = gbuf[:, gj % GB_BUFS, off:off + K, :].rearrange("p k f -> p f k")
                            v.tensor_reduce(tmp[:], seg, axis=mybir.AxisListType.X, op=OP.add)
                            ta = v.tensor_add(agg[:, t, :], agg[:, t, :], tmp[:])
                            if layer == 1 and c == NCLS - 1:
                                ta.then_inc(s_z)
                        v.nop().then_inc(s_red, 1)
                    if layer == 0:
                        # x2 = relu(dinv * agg), then pipelined transpose-copy/ep2
                        for t in gtiles:
                            v.tensor_scalar(out=agg[:, t, :], in0=agg[:, t, :],
                                            scalar1=dvc_sb[:, t:t + 1], scalar2=0.0,
                                            op0=OP.mult, op1=OP.max).then_inc(s_x2)
                        for t in gtiles:
                            v.wait_ge(s_tp, t + 1)
                            v.tensor_copy(x2T[:NHID, t % 2, :], pT[:, t % 2, :128]).then_inc(s_cp)
                            if t >= 1:
                                _ep2(t - 1)
                    else:
                        # final: out = dinv*agg - ln(sum exp), one group behind
                        # so the Act exp/Ln round trip hides under gathers
                        if gi >= 1:
                            _final(gi - 1)
                if layer == 0:
                    _ep2(NT - 1)
                else:
                    _final(len(groups) - 1)

        # ---------------- scalar engine: exp accumulate + per-group Ln ----------------
        @block.scalar
        def _(sc):
            sc.wait_ge(s_in, 48)
            for gi, gtiles in enumerate(groups):
                for t in gtiles:
                    sc.wait_ge(s_z, t + 1)
                    sc.activation(tmp2[:], agg[:, t, :NCLASS], AF.Exp,
                                  scale=dvc_sb[:, t:t + 1],
                                  accum_out=lse[:, t:t + 1])
                t0, t1 = gtiles[0], gtiles[-1] + 1
                sc.activation(lnl[:, t0:t1], lse[:, t0:t1], AF.Ln).then_inc(s_ln)

    nc.compile()
    return nc


_LAST_NC = None


def kernel(x, W1, b1, W2, b2, edge_index):
    global _LAST_NC
    from concourse.bass_utils import run_bass_kernel_spmd

    x = np.asarray(x)
    W1 = np.asarray(W1); b1 = np.asarray(b1)
    W2 = np.asarray(W2); b2 = np.asarray(b2)
    edge_index = np.asarray(edge_index)

    meta, streams = _host_prep(edge_index)
    calls = meta["calls"]
    groups = meta["groups"]
    nc = _build_program(calls, groups)
    _LAST_NC = nc

    dinv = meta["dinv"]
    blocks = meta["blocks"]
    ident = np.eye(128, dtype=np.float32)
    w1t_np = W1.T.astype(np.float32).copy()                      # [128,64]
    w2p_np = np.zeros((NHID + 1, F), np.float32)
    w2p_np[:NHID, :NCLASS] = W2.T
    w2p_np[NHID, :NCLASS] = b2                                   # bias row
    b1b_np = np.tile(b1.astype(np.float32), (128, 4))

    in_maps = []
    for k in range(NCORES):
        blk = blocks[k]
        real = blk >= 0
        xTk = np.zeros((128, XCOLS), np.float32)
        dvk = np.zeros(XCOLS, np.float32)
        idxs = np.flatnonzero(real)
        xcols = np.zeros((XCOLS, NFEAT), np.float32)
        xcols[idxs] = x[blk[idxs]]
        xTk[:, :] = xcols.T
        dvk[idxs] = dinv[blk[idxs]]
        dvc_np = dvk.reshape(NT1, 128).T.copy()                  # [128, NT1]
        in_maps.append({
            "xT": xTk, "idx": streams[k], "w1t": w1t_np, "w2p": w2p_np,
            "b1b": b1b_np, "dvc": dvc_np, "idn": ident,
            "zro": np.zeros((1, F), np.float32),
        })

    res = run_bass_kernel_spmd(nc, in_maps, list(range(NCORES)))

    out = np.empty((N, NCLASS), np.float32)
    for k in range(NCORES):
        blk = blocks[k]
        real = blk >= 0
        out[blk[real]] = res.results[k]["out"][np.flatnonzero(real)]
    return out


# revision 38
# speedup vs baseline: 1.1280x; 1.0607x over previous
"""2-layer GCN (gnn_message_passing) on 8 Trainium2 NeuronCores.

Strategy (v2):
  - Fold the symmetric degree normalization into per-node scaling:
      msg_e = dinv[src]*dinv[dst]*h[src]  =>  agg = dinv * A_sum(dinv * h)
    so aggregation is a pure unweighted gather + segment-sum.
  - Node-partition the graph over 8 cores (12544 dst nodes per core).
  - Each core computes h' = (x_shard @ W^T + b) * dinv for its shard,
    AllGathers the full h' table (f32 rows of 64 floats = 256B), then
    aggregates its own dst tiles with batched dma_gather (int16 indices
    -> 4 table "classes" of <=25090 rows each) + strided vector reduces.
  - Self-loop terms never enter the gather streams: each dst tile's
    accumulator is initialized with dinv^2 * h from the SBUF-resident
    h' tiles.
  - Dst nodes are clustered into tiles by their (max, argmax, full
    vector) in-neighbor class profile so the per-(tile,class) padded
    slot count tracks the actual in-degree profile (~22% padding).
  - Layer-1 epilogue (relu fused into the dinv scale, transpose,
    matmul2 with bias folded in via a ones row) is pipelined per tile
    group under the remaining gathers, so AllGather-2 launches right
    after the last layer-1 reduce. The layer-2 softmax pipeline (exp
    with accumulate, one deferred Ln over all tiles, fused
    subtract+store) runs under the layer-2 gathers.
"""

import sys
import numpy as np

sys.path.insert(0, "/opt/trn_rl_repo")

N = 100000
E = 1600000
NFEAT, NHID, NCLASS = 128, 64, 40
NCORES = 8
CPN = 12544            # dst nodes per core (98 tiles of 128)
BLK = CPN + 1          # AllGather block rows per core (+1 zero pad row)
NT = CPN // 128        # 98 aggregation tiles per core
NT1 = NT + 1           # 99 phase-1 tiles (covers the pad row)
XCOLS = NT1 * 128      # 12672
NCLS = 4
CLS_ROWS = 2 * BLK     # 25090 table rows per class (= 2 core blocks)
TBL = NCORES * BLK     # 100360
PAD_LOCAL = CPN        # local index of the zero row within a class
GB_SLOTS = 32          # gather ring buffer slots per call
GB_BUFS = 8
ORING = 14             # output store ring (2 batches of 7 tiles)
F = 64                 # table row width (f32) = 256B
NXCH = 4               # xT upload chunks


def _host_prep(edge_index):
    """Returns permutation/class layout + per-core padded gather streams."""
    src = edge_index[0].astype(np.int64)
    dst = edge_index[1].astype(np.int64)
    deg = np.bincount(src, minlength=N) + 1      # +1: self-loop
    dinv = (1.0 / np.sqrt(deg.astype(np.float64))).astype(np.float32)

    # ---- greedy class assignment of sources (balance each dst's in-nbrs) ----
    order_e = np.argsort(src, kind="stable")
    d_sorted = dst[order_e]
    sptr = np.searchsorted(src[order_e], np.arange(N + 1))
    cap = NCORES * CPN // NCLS                    # 25088 real nodes max per class
    cnt = np.zeros((N, NCLS), np.int32)
    cls = np.full(N, -1, np.int8)
    szs = np.zeros(NCLS, np.int64)
    outdeg = np.bincount(src, minlength=N)
    sorder = np.argsort(-outdeg, kind="stable")
    for s in sorder:
        dd = d_sorted[sptr[s]:sptr[s + 1]]
        sc = (4.0 ** cnt[dd, :]).sum(0)
        sc = sc + (szs >= cap) * 1e30
        c = int(sc.argmin())
        cls[s] = c
        szs[c] += 1
        cnt[dd, c] += 1

    # ---- node -> (core, position): cluster similar in-profiles per tile ----
    # (the greedy's cnt undercounts parallel edges due to fancy-index
    # buffering; recompute exactly for the layout sort)
    cnt = np.zeros((N, NCLS), np.int32)
    np.add.at(cnt, (dst, cls[src]), 1)
    blocks = []                                   # per core: array of node ids (-1 = dummy)
    for c in range(NCLS):
        nodes_c = np.flatnonzero(cls == c)
        cc = cnt[nodes_c]
        order = np.lexsort((cc[:, 3], cc[:, 2], cc[:, 1], cc[:, 0],
                            cc.argmax(1), cc.max(1)))
        nodes_c = nodes_c[order]
        a = np.full(CPN, -1, np.int64)
        b = np.full(CPN, -1, np.int64)
        a[: (len(nodes_c) + 1) // 2] = nodes_c[0::2]
        b[: len(nodes_c) // 2] = nodes_c[1::2]
        blocks.append(a)
        blocks.append(b)

    # table row of node n (within the AllGathered table)
    row = np.full(N, -1, np.int64)
    for k in range(NCORES):
        blk = blocks[k]
        real = blk >= 0
        row[blk[real]] = k * BLK + np.flatnonzero(real)

    # ---- per-(core,tile,class,partition) in-neighbor counts ----
    dcore = np.empty(N, np.int64)
    dpos = np.empty(N, np.int64)
    for k in range(NCORES):
        blk = blocks[k]
        real = blk >= 0
        dcore[blk[real]] = k
        dpos[blk[real]] = np.flatnonzero(real)
    ecore = dcore[dst]
    epos = dpos[dst]
    etile = epos // 128
    epart = epos % 128
    ecls = cls[src].astype(np.int64)
    esrow = row[src] - ecls * CLS_ROWS            # class-local table row (0..25089)
    assert esrow.min() >= 0 and esrow.max() < CLS_ROWS

    key = ((ecore * NT + etile) * NCLS + ecls) * 128 + epart
    eorder = np.argsort(key, kind="stable")
    key_s = key[eorder]
    esrow_s = esrow[eorder]
    counts = np.bincount(key_s, minlength=NCORES * NT * NCLS * 128)
    counts = counts.reshape(NCORES, NT, NCLS, 128)
    kmax = counts.max(axis=(0, 3))                # common K per (tile,class), max over cores
    kmax = np.maximum(kmax, 1)                    # avoid zero-length segments

    # ---- call grouping: consecutive tiles, per-class slot sum <= GB_SLOTS ----
    groups = []
    cur = []
    for t in range(NT):
        trial = cur + [t]
        if cur and max(kmax[trial, c].sum() for c in range(NCLS)) > GB_SLOTS:
            groups.append(cur)
            cur = [t]
        else:
            cur = trial
        if kmax[t].max() > GB_SLOTS:
            raise RuntimeError("single tile exceeds gather buffer")
    groups.append(cur)
    if len(groups[-1]) > 3:                       # small last group -> short tail
        groups.append(groups[-1][-3:])
        groups[-2] = groups[-2][:-3]

    # calls: (class, tiles, seg_offsets, nslots); same order for both layers
    calls = []
    for g in groups:
        for c in range(NCLS):
            offs = np.concatenate([[0], np.cumsum(kmax[g, c])])
            calls.append((c, list(g), offs[:-1].tolist(), int(offs[-1])))

    total_slots = sum(nsl for (_, _, _, nsl) in calls)

    # ---- build per-core int16 index stream in call order (vectorized) ----
    flat_counts = counts.reshape(-1)
    starts = np.concatenate([[0], np.cumsum(flat_counts)])[:-1].reshape(NCORES, NT, NCLS, 128)
    epart_s = epart[eorder]

    streams = []
    for k in range(NCORES):
        stream = np.full(total_slots * 128, PAD_LOCAL, np.int16)
        base = 0
        for (c, tiles, offs, nsl) in calls:
            for t, off in zip(tiles, offs):
                cnts = counts[k, t, c]            # [128]
                st = starts[k, t, c]              # [128]
                tot = int(cnts.sum())
                if tot:
                    sl = slice(st[0], st[0] + tot)
                    parts = epart_s[sl]
                    jj = np.arange(tot) - np.repeat(st - st[0], cnts)
                    stream[base + (off + jj) * 128 + parts] = esrow_s[sl].astype(np.int16)
            base += nsl * 128
        # wrap for dma_gather: idxs[p, j] = stream[j*16 + p%16]
        wrapped = stream.reshape(-1, 16).T        # [16, cols]
        streams.append(np.tile(wrapped, (8, 1)))  # [128, cols]

    meta = dict(blocks=blocks, row=row, dinv=dinv, calls=calls, groups=groups,
                total_slots=total_slots, kmax=kmax)
    return meta, streams


def _build_program(calls, groups):
    import concourse.bacc as bacc
    import concourse.bass as bass
    from concourse import mybir
    from concourse.library_config import mlp
    from contextlib import ExitStack

    AF = mybir.ActivationFunctionType
    OP = mybir.AluOpType
    nc = bacc.Bacc("TRN2", target_bir_lowering=False, debug=False)

    NCALLS = len(calls)
    total_slots = sum(nsl for (_, _, _, nsl) in calls)
    COLS = total_slots * 8                        # int16 idx cols per partition
    # tile-aligned upload chunk boundaries (in cols)
    tchunk = [0, 25, 50, 75, NT1]
    xbound = [t * 128 for t in tchunk]

    xT = nc.declare_dram_parameter("xT", [128, XCOLS], mybir.dt.float32, isOutput=False)
    idxp = nc.declare_dram_parameter("idx", [128, COLS], mybir.dt.int16, isOutput=False)
    w1t = nc.declare_dram_parameter("w1t", [128, NHID], mybir.dt.float32, isOutput=False)
    w2p = nc.declare_dram_parameter("w2p", [NHID + 1, F], mybir.dt.float32, isOutput=False)
    b1b = nc.declare_dram_parameter("b1b", [128, 4 * NHID], mybir.dt.float32, isOutput=False)
    dvc = nc.declare_dram_parameter("dvc", [128, NT1], mybir.dt.float32, isOutput=False)
    idn = nc.declare_dram_parameter("idn", [128, 128], mybir.dt.float32, isOutput=False)
    zro = nc.declare_dram_parameter("zro", [1, F], mybir.dt.float32, isOutput=False)
    outp = nc.declare_dram_parameter("out", [CPN, NCLASS], mybir.dt.float32, isOutput=True)

    h1_own = nc.dram_tensor("h1_own", [XCOLS, F], mybir.dt.float32)
    h2_own = nc.dram_tensor("h2_own", [BLK, F], mybir.dt.float32)
    h1_full = nc.dram_tensor("h1_full", [TBL, F], mybir.dt.float32, addr_space="Shared")
    h2_full = nc.dram_tensor("h2_full", [TBL, F], mybir.dt.float32, addr_space="Shared")

    with ExitStack() as stack:
        ec = stack.enter_context
        block = ec(nc.Block())
        xT_sb = ec(nc.sbuf_tensor("xT_sb", [128, XCOLS], mybir.dt.float32))
        idx_sb = ec(nc.sbuf_tensor("idx_sb", [128, COLS], mybir.dt.int16))
        w1t_sb = ec(nc.sbuf_tensor("w1t_sb", [128, NHID], mybir.dt.float32))
        w2p_sb = ec(nc.sbuf_tensor("w2p_sb", [NHID + 1, F], mybir.dt.float32))
        b1b_sb = ec(nc.sbuf_tensor("b1b_sb", [128, 4 * NHID], mybir.dt.float32))
        dvc_sb = ec(nc.sbuf_tensor("dvc_sb", [128, NT1], mybir.dt.float32))
        idn_sb = ec(nc.sbuf_tensor("idn_sb", [128, 128], mybir.dt.float32))
        zro_sb = ec(nc.sbuf_tensor("zro_sb", [1, F], mybir.dt.float32))
        gbuf = ec(nc.sbuf_tensor("gbuf", [128, GB_BUFS, GB_SLOTS, F], mybir.dt.float32))
        agg = ec(nc.sbuf_tensor("agg", [128, NT, F], mybir.dt.float32))
        hs = ec(nc.sbuf_tensor("hs", [128, NT1, F], mybir.dt.float32))
        x2T = ec(nc.sbuf_tensor("x2T", [NHID + 1, 2, 128], mybir.dt.float32))
        osb = ec(nc.sbuf_tensor("osb", [128, ORING, NCLASS], mybir.dt.float32))
        tmp = ec(nc.sbuf_tensor("tmp", [128, F], mybir.dt.float32))
        tmpn = ec(nc.sbuf_tensor("tmpn", [128, 16, F], mybir.dt.float32))
        tmp2 = ec(nc.sbuf_tensor("tmp2", [128, NCLASS], mybir.dt.float32))
        lse = ec(nc.sbuf_tensor("lse", [128, NT], mybir.dt.float32))
        lnl = ec(nc.sbuf_tensor("lnl", [128, NT], mybir.dt.float32))
        ph1 = ec(nc.psum_tensor("ph1", [128, 8, NHID], mybir.dt.float32))
        pT = ec(nc.psum_tensor("pT", [NHID, 2, 512], mybir.dt.float32))
        p2 = ec(nc.psum_tensor("p2", [128, 2, 512], mybir.dt.float32))
        sems = {n: ec(nc.semaphore(n)) for n in [
            "s_in", "s_xin", "s_idx", "s_mm1", "s_ep1", "s_st1", "s_cc", "s_g",
            "s_red", "s_x2", "s_tp", "s_cp", "s_mm2", "s_ep2", "s_st2", "s_z",
            "s_ln", "s_sm", "s_out"]}
        (s_in, s_xin, s_idx, s_mm1, s_ep1, s_st1, s_cc, s_g, s_red, s_x2, s_tp,
         s_cp, s_mm2, s_ep2, s_st2, s_z, s_ln, s_sm, s_out) = (
            sems[n] for n in ["s_in", "s_xin", "s_idx", "s_mm1", "s_ep1",
                              "s_st1", "s_cc", "s_g", "s_red", "s_x2", "s_tp",
                              "s_cp", "s_mm2", "s_ep2", "s_st2", "s_z", "s_ln",
                              "s_sm", "s_out"])

        # ---------------- sync engine: uploads + stores ----------------
        @block.sync
        def _(se: bass.BassEngine):
            se.dma_start(w1t_sb[:], w1t[:]).then_inc(s_in, 16)       # s_in 16
            se.dma_start(b1b_sb[:], b1b[:]).then_inc(s_in, 16)       # s_in 32
            se.dma_start(dvc_sb[:], dvc[:]).then_inc(s_in, 16)       # s_in 48
            for c in range(NXCH):
                se.dma_start(xT_sb[:, xbound[c]:xbound[c + 1]],
                             xT[:, xbound[c]:xbound[c + 1]]).then_inc(s_xin, 16)
            se.dma_start(idn_sb[:], idn[:]).then_inc(s_in, 16)       # s_in 64
            se.dma_start(w2p_sb[:], w2p[:]).then_inc(s_in, 16)       # s_in 80
            se.dma_start(zro_sb[:], zro[:]).then_inc(s_in, 16)       # s_in 96
            # h2 zero pad row (zro_sb upload must have landed first)
            se.wait_ge(s_in, 96)
            se.dma_start(h2_own[CPN:CPN + 1, :], zro_sb[:]).then_inc(s_st2, 16)
            # phase-1 stores, 9 tiles per DMA (99 = 11*9)
            for i in range(NT1 // 9):
                t = 9 * i
                se.wait_ge(s_ep1, t + 9)
                dst_ap = h1_own[t * 128:(t + 9) * 128, :].rearrange("(k p) f -> p k f", p=128)
                se.dma_start(dst_ap, hs[:, t:t + 9, :]).then_inc(s_st1, 16)
            # layer-2 h2' stores, 7 tiles per DMA (98 = 14*7)
            for i in range(NT // 7):
                t = 7 * i
                se.wait_ge(s_ep2, t + 7)
                dst_ap = h2_own[t * 128:(t + 7) * 128, :].rearrange("(k p) f -> p k f", p=128)
                se.dma_start(dst_ap, hs[:, t:t + 7, :]).then_inc(s_st2, 16)
            # output stores, 7 tiles per DMA
            for i in range(NT // 7):
                t = 7 * i
                se.wait_ge(s_sm, t + 7)
                dst_ap = outp[t * 128:(t + 7) * 128, :].rearrange("(k p) f -> p k f", p=128)
                se.dma_start(dst_ap, osb[:, t % ORING:t % ORING + 7, :]).then_inc(s_out, 16)
            se.wait_ge(s_out, 16 * (NT // 7))

        # ---------------- gpsimd: idx upload, collectives, gathers ----------------
        @block.gpsimd
        def _(g: bass.BassGpSimd):
            g.load_library(mlp)
            g.dma_start(idx_sb[:], idxp[:]).then_inc(s_idx, 16)
            g.wait_ge(s_idx, 16)
            for layer in (0, 1):
                tblt = h1_full if layer == 0 else h2_full
                if layer == 0:
                    g.wait_ge(s_st1, 16 * (NT1 // 9))
                    g.collective_compute(
                        "AllGather", mybir.AluOpType.bypass,
                        replica_groups=[list(range(NCORES))],
                        ins=[h1_own[0:BLK, :].opt()],
                        outs=[h1_full[:, :].opt()],
                    ).then_inc(s_cc)
                    g.wait_ge(s_cc, 1)
                else:
                    g.wait_ge(s_st2, 16 * (NT // 7 + 1))
                    g.collective_compute(
                        "AllGather", mybir.AluOpType.bypass,
                        replica_groups=[list(range(NCORES))],
                        ins=[h2_own[:, :].opt()],
                        outs=[h2_full[:, :].opt()],
                    ).then_inc(s_cc)
                    g.wait_ge(s_cc, 2)
                off = 0
                for j, (c, tiles, offs, nsl) in enumerate(calls):
                    gj = layer * NCALLS + j
                    if gj >= GB_BUFS:
                        g.wait_ge(s_red, gj - GB_BUFS + 1)
                    nidx = nsl * 128
                    g.dma_gather(
                        gbuf[:, gj % GB_BUFS, :nsl, :],
                        tblt[c * CLS_ROWS:(c + 1) * CLS_ROWS, :],
                        idx_sb[:, off * 8:(off + nsl) * 8],
                        nidx, nidx, F,
                        single_packet=False,
                    ).then_inc(s_g, 16)
                    off += nsl
                off = 0

        # ---------------- tensor engine ----------------
        @block.tensor
        def _(te):
            te.wait_ge(s_in, 16)
            for t in range(NT1):
                te.wait_ge(s_xin, 16 * (t // 25 + 1))
                if t >= 8:
                    te.wait_ge(s_ep1, t - 7)
                te.matmul(ph1[:, t % 8, :], xT_sb[:, t * 128:(t + 1) * 128], w1t_sb[:]).then_inc(s_mm1)
            # layer-2: software-pipelined transpose / matmul2
            te.wait_ge(s_in, 80)
            for t in range(NT):
                te.wait_ge(s_x2, t + 1)
                if t >= 2:
                    te.wait_ge(s_cp, t - 1)
                te.transpose(pT[:, t % 2, :128], agg[:, t, :], idn_sb[:]).then_inc(s_tp)
                if t >= 1:
                    te.wait_ge(s_cp, t)
                    if t >= 3:
                        te.wait_ge(s_ep2, t - 2)
                    te.matmul(p2[:, (t - 1) % 2, :F], x2T[:, (t - 1) % 2, :], w2p_sb[:]).then_inc(s_mm2)
            te.wait_ge(s_cp, NT)
            te.wait_ge(s_ep2, NT - 2)
            te.matmul(p2[:, (NT - 1) % 2, :F], x2T[:, (NT - 1) % 2, :], w2p_sb[:]).then_inc(s_mm2)

        # ---------------- vector engine ----------------
        @block.vector
        def _(v: bass.BassVectorEngine):
            v.wait_ge(s_in, 48)
            v.memset(x2T[NHID:NHID + 1, :, :], 1.0)   # bias row for matmul2
            # phase 1: h1' tiles (persistent); bias-add batched over 4 tiles
            b1b4 = b1b_sb[:].rearrange("p (k f) -> p k f", k=4)
            for t0 in range(0, NT1, 4):
                nb = min(4, NT1 - t0)
                v.wait_ge(s_mm1, t0 + nb)
                if nb == 4:
                    v.tensor_add(hs[:, t0:t0 + 4, :], ph1[:, t0 % 8:t0 % 8 + 4, :], b1b4)
                else:
                    for t in range(t0, t0 + nb):
                        v.tensor_add(hs[:, t, :], ph1[:, t % 8, :], b1b_sb[:, :NHID])
                for t in range(t0, t0 + nb):
                    v.tensor_scalar(out=hs[:, t, :], in0=hs[:, t, :],
                                    scalar1=dvc_sb[:, t:t + 1], scalar2=None,
                                    op0=OP.mult).then_inc(s_ep1)
            # self-loop agg init (runs under AllGather-1)
            for t in range(NT):
                v.tensor_scalar(out=agg[:, t, :], in0=hs[:, t, :],
                                scalar1=dvc_sb[:, t:t + 1], scalar2=None,
                                op0=OP.mult)

            def _final(gi):
                for t in groups[gi]:
                    v.wait_ge(s_ln, gi + 1)
                    if t >= ORING:
                        v.wait_ge(s_out, 16 * ((t - ORING) // 7 + 1))
                    v.tensor_scalar(out=osb[:, t % ORING, :], in0=agg[:, t, :NCLASS],
                                    scalar1=dvc_sb[:, t:t + 1], scalar2=lnl[:, t:t + 1],
                                    op0=OP.mult, op1=OP.subtract).then_inc(s_sm)

            def _ep2(t):
                # h2' tile + layer-2 self-loop agg init (overwrites x2 in agg)
                v.wait_ge(s_mm2, t + 1)
                v.tensor_scalar(out=hs[:, t, :], in0=p2[:, t % 2, :F],
                                scalar1=dvc_sb[:, t:t + 1], scalar2=None,
                                op0=OP.mult).then_inc(s_ep2)
                v.tensor_scalar(out=agg[:, t, :], in0=p2[:, t % 2, :F],
                                scalar1=dvc_sb[:, t:t + 1], scalar2=dvc_sb[:, t:t + 1],
                                op0=OP.mult, op1=OP.mult)

            for layer in (0, 1):
                for gi, gtiles in enumerate(groups):
                    for c in range(NCLS):
                        j = gi * NCLS + c
                        (_, tiles, offs, nsl) = calls[j]
                        gj = layer * NCALLS + j
                        v.wait_ge(s_g, 16 * (gj + 1))
                        # all reduces first, then release the gather buf,
                        # then merge into agg
                        for ti, t in enumerate(tiles):
                            off = offs[ti]
                            K = (offs[ti + 1] - offs[ti]) if ti + 1 < len(tiles) else nsl - offs[ti]
                            seg = gbuf[:, gj % GB_BUFS, off:off + K, :].rearrange("p k f -> p f k")
                            v.tensor_reduce(tmpn[:, ti, :], seg, axis=mybir.AxisListType.X, op=OP.add)
                        v.nop().then_inc(s_red, 1)
                        for ti, t in enumerate(tiles):
                            ta = v.tensor_add(agg[:, t, :], agg[:, t, :], tmpn[:, ti, :])
                            if layer == 1 and c == NCLS - 1:
                                ta.then_inc(s_z)
                    if layer == 0:
                        # x2 = relu(dinv * agg), then pipelined transpose-copy/ep2
                        for t in gtiles:
                            v.tensor_scalar(out=agg[:, t, :], in0=agg[:, t, :],
                                            scalar1=dvc_sb[:, t:t + 1], scalar2=0.0,
                                            op0=OP.mult, op1=OP.max).then_inc(s_x2)
                        for t in gtiles:
                            v.wait_ge(s_tp, t + 1)
                            v.tensor_copy(x2T[:NHID, t % 2, :], pT[:, t % 2, :128]).then_inc(s_cp)
                            if t >= 1:
                                _ep2(t - 1)
                    else:
                        # final: out = dinv*agg - ln(sum exp), one group behind
                        # so the Act exp/Ln round trip hides under gathers
                        if gi >= 1:
                            _final(gi - 1)
                if layer == 0:
                    _ep2(NT - 1)
                else:
                    _final(len(groups) - 1)

        # ---------------- scalar engine: exp accumulate + per-group Ln ----------------
        @block.scalar
        def _(sc):
            sc.wait_ge(s_in, 48)
            for gi, gtiles in enumerate(groups):
                for t in gtiles:
                    sc.wait_ge(s_z, t + 1)
                    sc.activation(tmp2[:], agg[:, t, :NCLASS], AF.Exp,
                                  scale=dvc_sb[:, t:t + 1],
                                  accum_out=lse[:, t:t + 1])
                t0, t1 = gtiles[0], gtiles[-1] + 1
                sc.activation(lnl[:, t0:t1], lse[:, t0:t1], AF.Ln).then_inc(s_ln)

    nc.compile()
    return nc


_LAST_NC = None


def kernel(x, W1, b1, W2, b2, edge_index):
    global _LAST_NC
    from concourse.bass_utils import run_bass_kernel_spmd

    x = np.asarray(x)
    W1 = np.asarray(W1); b1 = np.asarray(b1)
    W2 = np.asarray(W2); b2 = np.asarray(b2)
    edge_index = np.asarray(edge_index)

    meta, streams = _host_prep(edge_index)
    calls = meta["calls"]
    groups = meta["groups"]
    nc = _build_program(calls, groups)
    _LAST_NC = nc

    dinv = meta["dinv"]
    blocks = meta["blocks"]
    ident = np.eye(128, dtype=np.float32)
    w1t_np = W1.T.astype(np.float32).copy()                      # [128,64]
    w2p_np = np.zeros((NHID + 1, F), np.float32)
    w2p_np[:NHID, :NCLASS] = W2.T
    w2p_np[NHID, :NCLASS] = b2                                   # bias row
    b1b_np = np.tile(b1.astype(np.float32), (128, 4))

    in_maps = []
    for k in range(NCORES):
        blk = blocks[k]
        real = blk >= 0
        xTk = np.zeros((128, XCOLS), np.float32)
        dvk = np.zeros(XCOLS, np.float32)
        idxs = np.flatnonzero(real)
        xcols = np.zeros((XCOLS, NFEAT), np.float32)
        xcols[idxs] = x[blk[idxs]]
        xTk[:, :] = xcols.T
        dvk[idxs] = dinv[blk[idxs]]
        dvc_np = dvk.reshape(NT1, 128).T.copy()                  # [128, NT1]
        in_maps.append({
            "xT": xTk, "idx": streams[k], "w1t": w1t_np, "w2p": w2p_np,
            "b1b": b1b_np, "dvc": dvc_np, "idn": ident,
            "zro": np.zeros((1, F), np.float32),
        })

    res = run_bass_kernel_spmd(nc, in_maps, list(range(NCORES)))

    out = np.empty((N, NCLASS), np.float32)
    for k in range(NCORES):
        blk = blocks[k]
        real = blk >= 0
        out[blk[real]] = res.results[k]["out"][np.flatnonzero(real)]
    return out
